# revision 7
# baseline (speedup 1.0000x reference)
"""GVSL loss (NCC + MSE + smoothness) as a distributed Bass kernel on 8 TRN2 cores.

Sharding: batch(2) x depth-quarters(4) = 8 shards. Each core computes partial
sums for its 32-deep output slab (with 4-voxel halo for the 9^3 box filter);
the final scalar reductions happen on the host.
"""

import os
import sys

for _p in ("/opt/trn_rl_repo",):
    if _p not in sys.path:
        sys.path.insert(0, _p)

import numpy as np

import concourse.bass as bass
import concourse.tile as tile
from concourse import bacc, mybir
from concourse.bass_utils import run_bass_kernel_spmd

F32 = mybir.dt.float32
AF = mybir.ActivationFunctionType
ALU = mybir.AluOpType

HP = 128          # partitions (H axis)
W = 128
D_FULL = 128
DQ = 32           # output depths per core
D_IN = DQ + 8     # slab rows incl. halo
WPAD = 144        # 5 zeros | 128 data | 11 zeros
WOFF = 5
NCHUNK = 2
DC_OUT = DQ // NCHUNK          # 16
DC_IN = DC_OUT + 8             # 24
DCPAD = 32                     # 5 zeros | 24 data | 3 zeros
DPOFF = 5
FLOW_D = DQ + 1                # 33
WIN3 = 729.0

N_IN = D_IN * WPAD             # 5760
N_CHUNK_IN = DC_IN * WPAD      # 3456
N_CHUNK_HB = DC_IN * W         # 3072  (H-boxed compact, per chunk)
N_DPAD = W * DCPAD             # 4096
N_BOX = W * DC_OUT             # 2048
N_RECON = DQ * W               # 4096
N_FLOW_C = FLOW_D * W          # 4224

# Use the fast fp32 path on the PE for the band matmuls (float32r).
USE_F32R = os.environ.get("GVSL_F32R", "0") == "1"

# acc_all columns
COL_CC0 = 0          # cc sum, chunk 0 / chunk 1
COL_MSE = 2
COL_DX = 3           # +c, W-axis diffs (3 channels)
COL_DY = 6           # +c, H-axis diffs
COL_DZ = 9           # +c, D-axis diffs

_CACHE = {}


def _build_program():
    nc = bacc.Bacc("TRN2", target_bir_lowering=False, debug=False, num_devices=8)

    d_imgsA = nc.dram_tensor("imgsA", [HP, N_IN], F32, kind="ExternalInput").ap()
    d_warped = nc.dram_tensor("warped", [HP, N_IN], F32, kind="ExternalInput").ap()
    d_recon = nc.dram_tensor("recon", [HP, N_RECON], F32, kind="ExternalInput").ap()
    d_flow = nc.dram_tensor("flow", [HP, 3 * N_FLOW_C], F32, kind="ExternalInput").ap()
    d_bandp = nc.dram_tensor("bandp", [HP, HP], F32, kind="ExternalInput").ap()
    d_bandn = nc.dram_tensor("bandn", [HP, HP], F32, kind="ExternalInput").ap()
    d_out = nc.dram_tensor("out", [HP, 16], F32, kind="ExternalOutput").ap()

    with tile.TileContext(nc) as tc:
        with tc.tile_pool(name="persist", bufs=1) as pp:
            acc = pp.tile([HP, 16], F32, tag="acc", name="acc")[:]
            eps_ap = pp.tile([HP, 1], F32, tag="epsc", name="epsc")[:]
            nc.gpsimd.memset(eps_ap, 1e-5)
            bandp = pp.tile([HP, HP], F32, tag="bandp", name="bandp")[:]
            bandn = pp.tile([HP, HP], F32, tag="bandn", name="bandn")[:]
            inJ = pp.tile([HP, N_IN], F32, tag="inJ", name="inJ")[:]
            nc.sync.dma_start(out=bandp, in_=d_bandp)
            nc.sync.dma_start(out=bandn, in_=d_bandn)
            nc.sync.dma_start(out=inJ, in_=d_imgsA)
            inJ_r = inJ.rearrange("p (d w) -> p d w", w=WPAD)

            # ---------------- phase 1: smoothness + MSE ----------------
            with (
                tc.tile_pool(name="flowp", bufs=2) as fp,
                tc.tile_pool(name="diffp", bufs=2) as fdp,
                tc.tile_pool(name="reconp", bufs=1) as rp,
            ):
                recon = rp.tile([HP, N_RECON], F32, tag="recon", name="recon")[:]
                nc.sync.dma_start(out=recon, in_=d_recon)
                recon_r = recon.rearrange("p (d w) -> p d w", w=W)

                # MSE = sum((imgsA - recon)^2) over the interior slab
                a_int = inJ_r[:, 4 : 4 + DQ, WOFF : WOFF + W]
                mbuf = fdp.tile([HP, N_RECON], F32, tag="dbuf", name="dbuf")[:]
                mbuf_r = mbuf.rearrange("p (d w) -> p d w", w=W)
                nc.vector.tensor_sub(mbuf_r, a_int, recon_r)
                nc.scalar.activation(
                    mbuf, mbuf, AF.Square, accum_out=acc[:, COL_MSE : COL_MSE + 1]
                )

                d_flow_r = d_flow.rearrange("p (c d w) -> p c d w", c=3, w=W)
                for c in range(3):
                    fc = fp.tile([HP, N_FLOW_C], F32, tag="fc", name="fc")[:]
                    nc.sync.dma_start(out=fc, in_=d_flow_r[:, c].rearrange("p d w -> p (d w)"))
                    fcs = fp.tile([HP, N_FLOW_C], F32, tag="fcs", name="fcs")[:]
                    # same DRAM rows shifted down one partition (H+1)
                    nc.sync.dma_start(
                        out=fcs[0 : HP - 1, :],
                        in_=d_flow_r[1:HP, c].rearrange("p d w -> p (d w)"),
                    )
                    fc_r = fc.rearrange("p (d w) -> p d w", w=W)
                    fcs_r = fcs.rearrange("p (d w) -> p d w", w=W)

                    # W-axis diffs (innermost)
                    db = fdp.tile([HP, N_RECON], F32, tag="dbuf", name="dbuf")[:]
                    db_x = db.rearrange("p (d w) -> p d w", w=W)[:, :, 0 : W - 1]
                    nc.vector.tensor_sub(
                        db_x, fc_r[:, 0:DQ, 1:W], fc_r[:, 0:DQ, 0 : W - 1]
                    )
                    col = COL_DX + c
                    nc.scalar.activation(
                        db.rearrange("p (d w) -> p d w", w=W)[:, :, 0 : W - 1],
                        db.rearrange("p (d w) -> p d w", w=W)[:, :, 0 : W - 1],
                        AF.Square,
                        accum_out=acc[:, col : col + 1],
                    )

                    # H-axis diffs via the shifted copy
                    db = fdp.tile([HP, N_RECON], F32, tag="dbuf", name="dbuf")[:]
                    nc.vector.tensor_sub(
                        db[0 : HP - 1, :],
                        fcs_r[0 : HP - 1, 0:DQ, :].rearrange("p d w -> p (d w)"),
                        fc_r[0 : HP - 1, 0:DQ, :].rearrange("p d w -> p (d w)"),
                    )
                    col = COL_DY + c
                    nc.scalar.activation(
                        db[0 : HP - 1, :],
                        db[0 : HP - 1, :],
                        AF.Square,
                        accum_out=acc[0 : HP - 1, col : col + 1],
                    )

                    # D-axis diffs
                    db = fdp.tile([HP, N_RECON], F32, tag="dbuf", name="dbuf")[:]
                    nc.vector.tensor_sub(
                        db,
                        fc_r[:, 1 : DQ + 1, :].rearrange("p d w -> p (d w)"),
                        fc_r[:, 0:DQ, :].rearrange("p d w -> p (d w)"),
                    )
                    col = COL_DZ + c
                    nc.scalar.activation(
                        db, db, AF.Square, accum_out=acc[:, col : col + 1]
                    )

            # ---------------- phase 2: NCC ----------------
            with (
                tc.tile_pool(name="srcI", bufs=1) as sip,
                tc.tile_pool(name="prodp", bufs=1) as prp,
                tc.tile_pool(name="cump", bufs=1) as cup,
                tc.tile_pool(name="dpadp", bufs=1) as dpp,
                tc.tile_pool(name="boxp", bufs=1) as bxp,
                tc.tile_pool(name="scrp", bufs=1) as scp,
                tc.tile_pool(name="psum", bufs=4, space="PSUM") as psp,
            ):
                inI = sip.tile([HP, N_IN], F32, tag="inI", name="inI")[:]
                nc.sync.dma_start(out=inI, in_=d_warped)
                inI_r = inI.rearrange("p (d w) -> p d w", w=WPAD)

                if USE_F32R:
                    F32R = mybir.dt.float32r
                    bandp_mm = bandp.bitcast(F32R)
                    bandn_mm = bandn.bitcast(F32R)
                else:
                    bandp_mm = bandp
                    bandn_mm = bandn

                for ch in range(NCHUNK):
                    r0 = ch * DC_OUT  # first slab row of this chunk
                    Jc = inJ_r[:, r0 : r0 + DC_IN, :]
                    Ic = inI_r[:, r0 : r0 + DC_IN, :]
                    Jc2 = Jc.rearrange("p d w -> p (d w)")
                    Ic2 = Ic.rearrange("p d w -> p (d w)")

                    boxes = {}
                    for v in ("J", "I", "II", "JJ", "IJ"):
                        # source tensor for this volume
                        if v == "J":
                            src2 = Jc2
                        elif v == "I":
                            src2 = Ic2
                        else:
                            prod = prp.tile([HP, N_CHUNK_IN], F32, tag="prod", name="prod")[:]
                            if v == "II":
                                nc.scalar.activation(prod, Ic2, AF.Square)
                            elif v == "JJ":
                                nc.scalar.activation(prod, Jc2, AF.Square)
                            else:
                                nc.vector.tensor_mul(prod, Ic2, Jc2)
                            src2 = prod

                        # W-axis cumulative sum (box diff is fused into the
                        # H-box matmul below via the +/- band pair)
                        cum = cup.tile([HP, N_DPAD], F32, tag="cum", name="cum")[:]
                        nc.vector.tensor_tensor_scan(
                            cum[:, 0:N_CHUNK_IN],
                            src2,
                            src2,
                            0.0,
                            op0=ALU.add,
                            op1=ALU.bypass,
                        )
                        cum_r = cum[:, 0:N_CHUNK_IN].rearrange(
                            "p (d w) -> p d w", w=WPAD
                        )

                        # dpad: [p, (w, dp)] with dp innermost, zero pads
                        dpad = dpp.tile([HP, N_DPAD], F32, tag="dpad", name="dpad")[:]
                        dpad_r = dpad.rearrange("p (w dp) -> p w dp", dp=DCPAD)
                        nc.gpsimd.memset(dpad_r[:, :, 0:DPOFF], 0.0)
                        nc.gpsimd.memset(dpad_r[:, :, DPOFF + DC_IN : DCPAD], 0.0)

                        # H-box matmuls with fused W-box difference:
                        #   psum = band^T @ cum[.., w+9] - band^T @ cum[.., w+0]
                        for j in range(N_CHUNK_HB // 512):
                            dlo = 4 * j
                            ps = psp.tile([HP, 512], F32, tag="ps", name="ps")[:]
                            rhs9 = cum_r[:, dlo : dlo + 4, 9 : 9 + W]
                            rhs0 = cum_r[:, dlo : dlo + 4, 0:W]
                            if USE_F32R:
                                rhs9 = rhs9.bitcast(mybir.dt.float32r)
                                rhs0 = rhs0.bitcast(mybir.dt.float32r)
                            nc.tensor.matmul(
                                ps, bandp_mm, rhs9, start=True, stop=False
                            )
                            nc.tensor.matmul(
                                ps, bandn_mm, rhs0, start=False, stop=True
                            )
                            # evacuate into dpad, transposed to (w, dp)
                            ps_wd = ps.rearrange("p (s w) -> p w s", w=W)
                            nc.scalar.copy(
                                dpad_r[:, :, DPOFF + dlo : DPOFF + dlo + 4], ps_wd
                            )

                        # D-axis cumsum + diff -> final 9^3 box sums
                        cumd = cup.tile([HP, N_DPAD], F32, tag="cum", name="cum")[:]
                        nc.vector.tensor_tensor_scan(
                            cumd, dpad, dpad, 0.0, op0=ALU.add, op1=ALU.bypass
                        )
                        cumd_r = cumd.rearrange("p (w dp) -> p w dp", dp=DCPAD)
                        B = bxp.tile([HP, N_BOX], F32, tag=f"box{v}", name=f"box{v}")[:]
                        B_r = B.rearrange("p (w d) -> p w d", d=DC_OUT)
                        nc.vector.tensor_sub(
                            B_r,
                            cumd_r[:, :, 13 : 13 + DC_OUT],
                            cumd_r[:, :, 4 : 4 + DC_OUT],
                        )
                        boxes[v] = B

                    # ---- cc math on [128, 2048] box sums ----
                    BJ, BI = boxes["J"], boxes["I"]
                    BII, BJJ, BIJ = boxes["II"], boxes["JJ"], boxes["IJ"]
                    s1 = scp.tile([HP, N_BOX], F32, tag="s1", name="s1")[:]
                    s2 = scp.tile([HP, N_BOX], F32, tag="s2", name="s2")[:]
                    s3 = scp.tile([HP, N_BOX], F32, tag="s3", name="s3")[:]
                    s4 = scp.tile([HP, N_BOX], F32, tag="s4", name="s4")[:]
                    s5 = scp.tile([HP, N_BOX], F32, tag="s5", name="s5")[:]

                    nc.vector.tensor_mul(s1, BI, BJ)
                    nc.vector.scalar_tensor_tensor(
                        s2, s1, -1.0 / WIN3, BIJ, op0=ALU.mult, op1=ALU.add
                    )  # cross
                    nc.scalar.activation(s1, s2, AF.Square)       # cross^2
                    nc.scalar.activation(s2, s1, AF.Ln)           # ln(cross^2)
                    nc.scalar.activation(s3, BI, AF.Square)
                    nc.vector.scalar_tensor_tensor(
                        s4, s3, -1.0 / WIN3, BII, op0=ALU.mult, op1=ALU.add
                    )  # I_var
                    nc.scalar.activation(s3, BJ, AF.Square)
                    nc.vector.scalar_tensor_tensor(
                        s5, s3, -1.0 / WIN3, BJJ, op0=ALU.mult, op1=ALU.add
                    )  # J_var
                    nc.vector.tensor_mul(s3, s4, s5)              # I_var * J_var
                    nc.scalar.activation(s4, s3, AF.Ln, bias=eps_ap)
                    nc.vector.tensor_sub(s3, s2, s4)
                    col = COL_CC0 + ch
                    nc.scalar.activation(
                        s5, s3, AF.Exp, accum_out=acc[:, col : col + 1]
                    )

            nc.sync.dma_start(out=d_out, in_=acc)

    nc.compile()
    return nc


def _make_band() -> tuple[np.ndarray, np.ndarray]:
    k = np.arange(HP)
    band = (np.abs(k[:, None] - k[None, :]) <= 4).astype(np.float32)
    return band, -band


def _shard_inputs(imgsA, recon_A, warped_BA, flow_BA):
    bandp, bandn = _make_band()
    in_maps = []
    for core in range(8):
        b, q = divmod(core, 4)
        d0 = DQ * q

        def slab(vol):
            s = np.zeros((HP, D_IN, WPAD), np.float32)
            lo, hi = d0 - 4, d0 + DQ + 4
            clo, chi = max(lo, 0), min(hi, D_FULL)
            s[:, clo - lo : chi - lo, WOFF : WOFF + W] = np.ascontiguousarray(
                vol[clo:chi].transpose(1, 0, 2)
            )
            return s.reshape(HP, N_IN)

        rec = np.ascontiguousarray(
            recon_A[b, 0, d0 : d0 + DQ].transpose(1, 0, 2)
        ).reshape(HP, N_RECON)

        fl = np.empty((HP, 3, FLOW_D, W), np.float32)
        hi = min(d0 + FLOW_D, D_FULL)
        n = hi - d0
        fl[:, :, :n] = flow_BA[b, :, d0:hi].transpose(2, 0, 1, 3)
        if n < FLOW_D:
            fl[:, :, n:] = fl[:, :, n - 1 : n]

        in_maps.append(
            {
                "imgsA": slab(imgsA[b, 0]),
                "warped": slab(warped_BA[b, 0]),
                "recon": rec,
                "flow": np.ascontiguousarray(fl).reshape(HP, 3 * N_FLOW_C),
                "bandp": bandp,
                "bandn": bandn,
            }
        )
    return in_maps


def _install_profile_shim():
    """Wire up NTFF profiling under axon when antenv.axon_hooks is absent."""
    try:
        import antenv.axon_hooks  # noqa: F401

        return True
    except ImportError:
        pass
    import contextlib
    import ctypes
    import types

    so_path = "/opt/axon/libaxon_pjrt.so"
    if not os.path.exists(so_path):
        return False
    lib = ctypes.CDLL(so_path)
    if not hasattr(lib, "axon_start_nrt_profile"):
        return False
    lib.axon_start_nrt_profile.argtypes = [
        ctypes.POINTER(ctypes.c_int64),
        ctypes.c_size_t,
    ]
    lib.axon_start_nrt_profile.restype = ctypes.c_int64
    lib.axon_stop_nrt_profile.argtypes = [ctypes.c_char_p]
    lib.axon_stop_nrt_profile.restype = ctypes.c_int64

    @contextlib.contextmanager
    def _hook(output_dir, device_ids):
        import jax

        jax.devices()
        if device_ids:
            ids = (ctypes.c_int64 * len(device_ids))(*device_ids)
            rc = lib.axon_start_nrt_profile(ids, len(device_ids))
        else:
            rc = lib.axon_start_nrt_profile(None, 0)
        if rc != 0:
            raise RuntimeError(f"axon_start_nrt_profile rc={rc}")
        try:
            yield
        finally:
            n = lib.axon_stop_nrt_profile(str(output_dir).encode())
            print(f"ntff profile: {n} file(s) written to {output_dir}")

    mod = types.ModuleType("antenv.axon_hooks")
    mod.get_axon_ntff_profile_hook = lambda: _hook
    mod.set_axon_ntff_profile_hook = lambda h: None
    import antenv

    sys.modules["antenv.axon_hooks"] = mod
    antenv.axon_hooks = mod

    # keep profile artifacts local instead of uploading to fishnet
    import concourse.bass_utils as _bu

    _bu.upload_artifacts = lambda tmpdir: tmpdir
    return True


LAST_EXEC_NS = None
LAST_RESULTS = None


def kernel(imgsA, recon_A, warped_BA, flow_BA):
    global LAST_EXEC_NS, LAST_RESULTS
    if "nc" not in _CACHE:
        _CACHE["nc"] = _build_program()
    nc = _CACHE["nc"]

    in_maps = _shard_inputs(
        np.asarray(imgsA, np.float32),
        np.asarray(recon_A, np.float32),
        np.asarray(warped_BA, np.float32),
        np.asarray(flow_BA, np.float32),
    )
    trace = os.environ.get("GVSL_TRACE", "0") == "1"
    if trace:
        trace = _install_profile_shim()
    tmpdir = os.environ.get("GVSL_TRACE_DIR") or None
    res = run_bass_kernel_spmd(
        nc, in_maps, core_ids=list(range(8)), trace=trace, tmpdir=tmpdir
    )
    LAST_EXEC_NS = res.exec_time_ns
    LAST_RESULTS = res

    cc = mse = dx = dy = dz = 0.0
    for r in res.results:
        o = np.asarray(r["out"], np.float64)
        cc += o[:, COL_CC0].sum() + o[:, COL_CC0 + 1].sum()
        mse += o[:, COL_MSE].sum()
        dx += o[:, COL_DX : COL_DX + 3].sum()
        dy += o[: HP - 1, COL_DY : COL_DY + 3].sum()
        dz += o[:, COL_DZ : COL_DZ + 3].sum()

    n_vox = 2 * 1 * 128 * 128 * 128
    n_d = 2 * 3 * 127 * 128 * 128
    ncc_loss = 1.0 - cc / n_vox
    mse_loss = mse / n_vox
    smooth_loss = (dx / n_d + dy / n_d + dz / n_d) / 3.0
    return (
        np.float32(ncc_loss),
        np.float32(mse_loss),
        np.float32(smooth_loss),
    )


# revision 18
# speedup vs baseline: 1.5100x; 1.5100x over previous
"""GVSL loss (NCC + MSE + smoothness) as a distributed Bass kernel on 8 TRN2 cores.

Sharding: batch(2) x depth-quarters(4) = 8 shards. Each core computes partial
sums for its 32-deep output slab (with 4-voxel halo for the 9^3 box filter);
the final scalar reductions happen on the host.
"""

import os
import sys

for _p in ("/opt/trn_rl_repo",):
    if _p not in sys.path:
        sys.path.insert(0, _p)

import numpy as np

import concourse.bass as bass
import concourse.tile as tile
from concourse import bacc, mybir
from concourse.bass_utils import run_bass_kernel_spmd

F32 = mybir.dt.float32
AF = mybir.ActivationFunctionType
ALU = mybir.AluOpType

HP = 128          # partitions (H axis)
W = 128
D_FULL = 128
DQ = 32           # output depths per core
D_IN = DQ + 8     # slab rows incl. halo
WPAD = 144        # 5 zeros | 128 data | 11 zeros
WOFF = 5
NCHUNK = 2
DC_OUT = DQ // NCHUNK          # 16
DC_IN = DC_OUT + 8             # 24
DCPAD = 30                     # 5 zeros | 24 data | 1 zero
DPOFF = 5
FLOW_D = DQ + 1                # 33
WIN3 = 729.0

N_IN = D_IN * WPAD             # 5760
N_CHUNK_IN = DC_IN * WPAD      # 3456
N_CHUNK_HB = DC_IN * W         # 3072  (H-boxed compact, per chunk)
N_DPAD = W * DCPAD             # 4096
N_BOX = W * DC_OUT             # 2048
N_RECON = DQ * W               # 4096
N_FLOW_C = FLOW_D * W          # 4224

# Use the fast fp32 path on the PE for the band matmuls (float32r).
USE_F32R = os.environ.get("GVSL_F32R", "0") == "1"

# acc_all columns
COL_CC0 = 0          # cc sum, chunk 0 / chunk 1
COL_MSE = 2
COL_DX = 3           # +c, W-axis diffs (3 channels)
COL_DY = 6           # +c, H-axis diffs
COL_DZ = 9           # +c, D-axis diffs

_CACHE = {}


def _build_program():
    nc = bacc.Bacc("TRN2", target_bir_lowering=False, debug=False, num_devices=8)

    d_imgsA = nc.dram_tensor("imgsA", [HP, N_IN], F32, kind="ExternalInput").ap()
    d_warped = nc.dram_tensor("warped", [HP, N_IN], F32, kind="ExternalInput").ap()
    d_recon = nc.dram_tensor("recon", [HP, N_RECON], F32, kind="ExternalInput").ap()
    d_flow = nc.dram_tensor("flow", [HP, 3 * N_FLOW_C], F32, kind="ExternalInput").ap()
    d_bandp = nc.dram_tensor("bandp", [HP, HP], F32, kind="ExternalInput").ap()
    d_bandn = nc.dram_tensor("bandn", [HP, HP], F32, kind="ExternalInput").ap()
    d_bidiag = nc.dram_tensor("bidiag", [HP, HP - 1], F32, kind="ExternalInput").ap()
    d_out = nc.dram_tensor("out", [HP, 16], F32, kind="ExternalOutput").ap()

    with tile.TileContext(nc) as tc:
        with tc.tile_pool(name="persist", bufs=1) as pp:
            acc = pp.tile([HP, 16], F32, tag="acc", name="acc")[:]
            eps_ap = pp.tile([HP, 1], F32, tag="epsc", name="epsc")[:]
            nc.gpsimd.memset(eps_ap, 1e-5)
            bandp = pp.tile([HP, HP], F32, tag="bandp", name="bandp")[:]
            bandn = pp.tile([HP, HP], F32, tag="bandn", name="bandn")[:]
            bidiag = pp.tile([HP, HP - 1], F32, tag="bidiag", name="bidiag")[:]
            inJ = pp.tile([HP, N_IN], F32, tag="inJ", name="inJ")[:]
            nc.sync.dma_start(out=bandp, in_=d_bandp)
            nc.sync.dma_start(out=bandn, in_=d_bandn)
            nc.sync.dma_start(out=bidiag, in_=d_bidiag)
            nc.sync.dma_start(out=inJ, in_=d_imgsA)
            inJ_r = inJ.rearrange("p (d w) -> p d w", w=WPAD)

            # ---------------- phase 1: smoothness + MSE ----------------
            with (
                tc.tile_pool(name="flowp", bufs=2) as fp,
                tc.tile_pool(name="diffp", bufs=2) as fdp,
                tc.tile_pool(name="reconp", bufs=1) as rp,
                tc.tile_pool(name="fpsum", bufs=4, space="PSUM") as fpsp,
            ):
                recon = rp.tile([HP, N_RECON], F32, tag="recon", name="recon")[:]
                nc.sync.dma_start(out=recon, in_=d_recon)
                recon_r = recon.rearrange("p (d w) -> p d w", w=W)

                # MSE = sum((imgsA - recon)^2) over the interior slab
                a_int = inJ_r[:, 4 : 4 + DQ, WOFF : WOFF + W]
                mbuf = fdp.tile([HP, N_RECON], F32, tag="dbuf", name="dbuf")[:]
                mbuf_r = mbuf.rearrange("p (d w) -> p d w", w=W)
                nc.vector.tensor_sub(mbuf_r, a_int, recon_r)
                nc.scalar.activation(
                    mbuf, mbuf, AF.Square, accum_out=acc[:, COL_MSE : COL_MSE + 1]
                )

                d_flow_r = d_flow.rearrange("p (c d w) -> p c d w", c=3, w=W)
                for c in range(3):
                    fc = fp.tile([HP, N_FLOW_C], F32, tag="fc", name="fc")[:]
                    nc.sync.dma_start(out=fc, in_=d_flow_r[:, c].rearrange("p d w -> p (d w)"))
                    fc_r = fc.rearrange("p (d w) -> p d w", w=W)

                    # W-axis diffs (innermost)
                    db = fdp.tile([HP, N_RECON], F32, tag="dbuf", name="dbuf")[:]
                    db_x = db.rearrange("p (d w) -> p d w", w=W)[:, :, 0 : W - 1]
                    nc.vector.tensor_sub(
                        db_x, fc_r[:, 0:DQ, 1:W], fc_r[:, 0:DQ, 0 : W - 1]
                    )
                    col = COL_DX + c
                    nc.scalar.activation(
                        db.rearrange("p (d w) -> p d w", w=W)[:, :, 0 : W - 1],
                        db.rearrange("p (d w) -> p d w", w=W)[:, :, 0 : W - 1],
                        AF.Square,
                        accum_out=acc[:, col : col + 1],
                    )

                    # H-axis diffs on the PE: psum = bidiag^T @ fc
                    #   out[m, :] = fc[m+1, :] - fc[m, :]
                    db = fdp.tile([HP, N_RECON], F32, tag="dbuf", name="dbuf")[:]
                    fc_flat = fc_r[:, 0:DQ, :].rearrange("p d w -> p (d w)")
                    for j in range(N_RECON // 512):
                        ps = fpsp.tile([HP, 512], F32, tag="fps", name="fps")[:]
                        nc.tensor.matmul(
                            ps[0 : HP - 1, :],
                            bidiag,
                            fc_flat[:, 512 * j : 512 * (j + 1)],
                            start=True,
                            stop=True,
                        )
                        nc.scalar.copy(
                            db[0 : HP - 1, 512 * j : 512 * (j + 1)], ps[0 : HP - 1, :]
                        )
                    col = COL_DY + c
                    nc.scalar.activation(
                        db[0 : HP - 1, :],
                        db[0 : HP - 1, :],
                        AF.Square,
                        accum_out=acc[0 : HP - 1, col : col + 1],
                    )

                    # D-axis diffs
                    db = fdp.tile([HP, N_RECON], F32, tag="dbuf", name="dbuf")[:]
                    nc.vector.tensor_sub(
                        db,
                        fc_r[:, 1 : DQ + 1, :].rearrange("p d w -> p (d w)"),
                        fc_r[:, 0:DQ, :].rearrange("p d w -> p (d w)"),
                    )
                    col = COL_DZ + c
                    nc.scalar.activation(
                        db, db, AF.Square, accum_out=acc[:, col : col + 1]
                    )

            # ---------------- phase 2: NCC ----------------
            with (
                tc.tile_pool(name="srcI", bufs=1) as sip,
                tc.tile_pool(name="prodp", bufs=1) as prp,
                tc.tile_pool(name="cump", bufs=2) as cup,
                tc.tile_pool(name="dpadp", bufs=2) as dpp,
                tc.tile_pool(name="boxp", bufs=1) as bxp,
                tc.tile_pool(name="scrp", bufs=1) as scp,
                tc.tile_pool(name="psum", bufs=6, space="PSUM") as psp,
            ):
                inI = sip.tile([HP, N_IN], F32, tag="inI", name="inI")[:]
                nc.sync.dma_start(out=inI, in_=d_warped)
                inI_r = inI.rearrange("p (d w) -> p d w", w=WPAD)

                if USE_F32R:
                    F32R = mybir.dt.float32r
                    bandp_mm = bandp.bitcast(F32R)
                    bandn_mm = bandn.bitcast(F32R)
                else:
                    bandp_mm = bandp
                    bandn_mm = bandn

                for ch in range(NCHUNK):
                    r0 = ch * DC_OUT  # first slab row of this chunk
                    Jc = inJ_r[:, r0 : r0 + DC_IN, :]
                    Ic = inI_r[:, r0 : r0 + DC_IN, :]
                    Jc2 = Jc.rearrange("p d w -> p (d w)")
                    Ic2 = Ic.rearrange("p d w -> p (d w)")

                    boxes = {}
                    for v in ("J", "I", "II", "JJ", "IJ"):
                        # source tensor for this volume
                        if v == "J":
                            src2 = Jc2
                        elif v == "I":
                            src2 = Ic2
                        else:
                            prod = prp.tile([HP, N_CHUNK_IN], F32, tag="prod", name="prod")[:]
                            if v == "II":
                                nc.scalar.activation(prod, Ic2, AF.Square)
                            elif v == "JJ":
                                nc.scalar.activation(prod, Jc2, AF.Square)
                            else:
                                nc.vector.tensor_mul(prod, Ic2, Jc2)
                            src2 = prod

                        # W-axis cumulative sum (box diff is fused into the
                        # H-box matmul below via the +/- band pair)
                        cum = cup.tile([HP, N_DPAD], F32, tag="cum", name="cum")[:]
                        nc.vector.tensor_tensor_scan(
                            cum[:, 0:N_CHUNK_IN],
                            src2,
                            src2,
                            0.0,
                            op0=ALU.add,
                            op1=ALU.bypass,
                        )
                        cum_r = cum[:, 0:N_CHUNK_IN].rearrange(
                            "p (d w) -> p d w", w=WPAD
                        )

                        # dpad: [p, (w, dp)] with dp innermost, zero pads
                        dpad = dpp.tile([HP, N_DPAD], F32, tag="dpad", name="dpad")[:]
                        dpad_r = dpad.rearrange("p (w dp) -> p w dp", dp=DCPAD)
                        nc.gpsimd.memset(dpad_r[:, :, 0:DPOFF], 0.0)
                        nc.gpsimd.memset(dpad_r[:, :, DPOFF + DC_IN : DCPAD], 0.0)

                        # H-box matmuls with fused W-box difference:
                        #   psum = band^T @ cum[.., w+9] - band^T @ cum[.., w+0]
                        for j in range(N_CHUNK_HB // 512):
                            dlo = 4 * j
                            ps = psp.tile([HP, 512], F32, tag="ps", name="ps")[:]
                            rhs9 = cum_r[:, dlo : dlo + 4, 9 : 9 + W]
                            rhs0 = cum_r[:, dlo : dlo + 4, 0:W]
                            if USE_F32R:
                                rhs9 = rhs9.bitcast(mybir.dt.float32r)
                                rhs0 = rhs0.bitcast(mybir.dt.float32r)
                            nc.tensor.matmul(
                                ps, bandp_mm, rhs9, start=True, stop=False
                            )
                            nc.tensor.matmul(
                                ps, bandn_mm, rhs0, start=False, stop=True
                            )
                            # evacuate into dpad, transposed to (w, dp)
                            ps_wd = ps.rearrange("p (s w) -> p w s", w=W)
                            nc.scalar.copy(
                                dpad_r[:, :, DPOFF + dlo : DPOFF + dlo + 4], ps_wd
                            )

                        # D-axis cumsum + diff -> final 9^3 box sums
                        cumd = cup.tile([HP, N_DPAD], F32, tag="cum", name="cum")[:]
                        nc.vector.tensor_tensor_scan(
                            cumd, dpad, dpad, 0.0, op0=ALU.add, op1=ALU.bypass
                        )
                        cumd_r = cumd.rearrange("p (w dp) -> p w dp", dp=DCPAD)
                        B = bxp.tile([HP, N_BOX], F32, tag=f"box{v}", name=f"box{v}")[:]
                        B_r = B.rearrange("p (w d) -> p w d", d=DC_OUT)
                        nc.vector.tensor_sub(
                            B_r,
                            cumd_r[:, :, 13 : 13 + DC_OUT],
                            cumd_r[:, :, 4 : 4 + DC_OUT],
                        )
                        boxes[v] = B

                    # ---- cc math on [128, 2048] box sums ----
                    BJ, BI = boxes["J"], boxes["I"]
                    BII, BJJ, BIJ = boxes["II"], boxes["JJ"], boxes["IJ"]
                    s1 = scp.tile([HP, N_BOX], F32, tag="s1", name="s1")[:]
                    s2 = scp.tile([HP, N_BOX], F32, tag="s2", name="s2")[:]
                    s3 = scp.tile([HP, N_BOX], F32, tag="s3", name="s3")[:]

                    nc.vector.tensor_mul(s1, BI, BJ)
                    nc.vector.scalar_tensor_tensor(
                        s2, s1, -1.0 / WIN3, BIJ, op0=ALU.mult, op1=ALU.add
                    )  # cross
                    nc.scalar.activation(s1, s2, AF.Square)       # cross^2
                    nc.scalar.activation(s2, s1, AF.Ln)           # ln(cross^2)
                    nc.scalar.activation(s1, BI, AF.Square)
                    nc.vector.scalar_tensor_tensor(
                        s3, s1, -1.0 / WIN3, BII, op0=ALU.mult, op1=ALU.add
                    )  # I_var
                    nc.scalar.activation(s1, BJ, AF.Square)
                    nc.vector.scalar_tensor_tensor(
                        BII, s1, -1.0 / WIN3, BJJ, op0=ALU.mult, op1=ALU.add
                    )  # J_var (overwrites dead BII)
                    nc.vector.tensor_mul(s1, s3, BII)             # I_var * J_var
                    nc.scalar.activation(s3, s1, AF.Ln, bias=eps_ap)
                    nc.vector.tensor_sub(s1, s2, s3)
                    col = COL_CC0 + ch
                    nc.scalar.activation(
                        s3, s1, AF.Exp, accum_out=acc[:, col : col + 1]
                    )

            nc.sync.dma_start(out=d_out, in_=acc)

    nc.compile()
    return nc


def _make_band() -> tuple[np.ndarray, np.ndarray, np.ndarray]:
    k = np.arange(HP)
    band = (np.abs(k[:, None] - k[None, :]) <= 4).astype(np.float32)
    m = np.arange(HP - 1)
    bidiag = np.zeros((HP, HP - 1), np.float32)
    bidiag[m + 1, m] = 1.0
    bidiag[m, m] = -1.0
    return band, -band, bidiag


def _shard_inputs(imgsA, recon_A, warped_BA, flow_BA):
    bandp, bandn, bidiag = _make_band()
    in_maps = []
    for core in range(8):
        b, q = divmod(core, 4)
        d0 = DQ * q

        def slab(vol):
            s = np.zeros((HP, D_IN, WPAD), np.float32)
            lo, hi = d0 - 4, d0 + DQ + 4
            clo, chi = max(lo, 0), min(hi, D_FULL)
            s[:, clo - lo : chi - lo, WOFF : WOFF + W] = np.ascontiguousarray(
                vol[clo:chi].transpose(1, 0, 2)
            )
            return s.reshape(HP, N_IN)

        rec = np.ascontiguousarray(
            recon_A[b, 0, d0 : d0 + DQ].transpose(1, 0, 2)
        ).reshape(HP, N_RECON)

        fl = np.empty((HP, 3, FLOW_D, W), np.float32)
        hi = min(d0 + FLOW_D, D_FULL)
        n = hi - d0
        fl[:, :, :n] = flow_BA[b, :, d0:hi].transpose(2, 0, 1, 3)
        if n < FLOW_D:
            fl[:, :, n:] = fl[:, :, n - 1 : n]

        in_maps.append(
            {
                "imgsA": slab(imgsA[b, 0]),
                "warped": slab(warped_BA[b, 0]),
                "recon": rec,
                "flow": np.ascontiguousarray(fl).reshape(HP, 3 * N_FLOW_C),
                "bandp": bandp,
                "bandn": bandn,
                "bidiag": bidiag,
            }
        )
    return in_maps


def _install_profile_shim():
    """Wire up NTFF profiling under axon when antenv.axon_hooks is absent."""
    try:
        import antenv.axon_hooks  # noqa: F401

        return True
    except ImportError:
        pass
    import contextlib
    import ctypes
    import types

    so_path = "/opt/axon/libaxon_pjrt.so"
    if not os.path.exists(so_path):
        return False
    lib = ctypes.CDLL(so_path)
    if not hasattr(lib, "axon_start_nrt_profile"):
        return False
    lib.axon_start_nrt_profile.argtypes = [
        ctypes.POINTER(ctypes.c_int64),
        ctypes.c_size_t,
    ]
    lib.axon_start_nrt_profile.restype = ctypes.c_int64
    lib.axon_stop_nrt_profile.argtypes = [ctypes.c_char_p]
    lib.axon_stop_nrt_profile.restype = ctypes.c_int64

    @contextlib.contextmanager
    def _hook(output_dir, device_ids):
        import jax

        jax.devices()
        if device_ids:
            ids = (ctypes.c_int64 * len(device_ids))(*device_ids)
            rc = lib.axon_start_nrt_profile(ids, len(device_ids))
        else:
            rc = lib.axon_start_nrt_profile(None, 0)
        if rc != 0:
            raise RuntimeError(f"axon_start_nrt_profile rc={rc}")
        try:
            yield
        finally:
            n = lib.axon_stop_nrt_profile(str(output_dir).encode())
            print(f"ntff profile: {n} file(s) written to {output_dir}")

    mod = types.ModuleType("antenv.axon_hooks")
    mod.get_axon_ntff_profile_hook = lambda: _hook
    mod.set_axon_ntff_profile_hook = lambda h: None
    import antenv

    sys.modules["antenv.axon_hooks"] = mod
    antenv.axon_hooks = mod

    # keep profile artifacts local instead of uploading to fishnet
    import concourse.bass_utils as _bu

    _bu.upload_artifacts = lambda tmpdir: tmpdir
    return True


LAST_EXEC_NS = None
LAST_RESULTS = None


def kernel(imgsA, recon_A, warped_BA, flow_BA):
    global LAST_EXEC_NS, LAST_RESULTS
    if "nc" not in _CACHE:
        _CACHE["nc"] = _build_program()
    nc = _CACHE["nc"]

    in_maps = _shard_inputs(
        np.asarray(imgsA, np.float32),
        np.asarray(recon_A, np.float32),
        np.asarray(warped_BA, np.float32),
        np.asarray(flow_BA, np.float32),
    )
    trace = os.environ.get("GVSL_TRACE", "0") == "1"
    if trace:
        trace = _install_profile_shim()
    tmpdir = os.environ.get("GVSL_TRACE_DIR") or None
    res = run_bass_kernel_spmd(
        nc, in_maps, core_ids=list(range(8)), trace=trace, tmpdir=tmpdir
    )
    LAST_EXEC_NS = res.exec_time_ns
    LAST_RESULTS = res

    cc = mse = dx = dy = dz = 0.0
    for r in res.results:
        o = np.asarray(r["out"], np.float64)
        cc += o[:, COL_CC0].sum() + o[:, COL_CC0 + 1].sum()
        mse += o[:, COL_MSE].sum()
        dx += o[:, COL_DX : COL_DX + 3].sum()
        dy += o[: HP - 1, COL_DY : COL_DY + 3].sum()
        dz += o[:, COL_DZ : COL_DZ + 3].sum()

    n_vox = 2 * 1 * 128 * 128 * 128
    n_d = 2 * 3 * 127 * 128 * 128
    ncc_loss = 1.0 - cc / n_vox
    mse_loss = mse / n_vox
    smooth_loss = (dx / n_d + dy / n_d + dz / n_d) / 3.0
    return (
        np.float32(ncc_loss),
        np.float32(mse_loss),
        np.float32(smooth_loss),
    )


# revision 19
# speedup vs baseline: 1.5222x; 1.0081x over previous
"""GVSL loss (NCC + MSE + smoothness) as a distributed Bass kernel on 8 TRN2 cores.

Sharding: batch(2) x depth-quarters(4) = 8 shards. Each core computes partial
sums for its 32-deep output slab (with 4-voxel halo for the 9^3 box filter);
the final scalar reductions happen on the host.
"""

import os
import sys

for _p in ("/opt/trn_rl_repo",):
    if _p not in sys.path:
        sys.path.insert(0, _p)

import numpy as np

import concourse.bass as bass
import concourse.tile as tile
from concourse import bacc, mybir
from concourse.bass_utils import run_bass_kernel_spmd

F32 = mybir.dt.float32
AF = mybir.ActivationFunctionType
ALU = mybir.AluOpType

HP = 128          # partitions (H axis)
W = 128
D_FULL = 128
DQ = 32           # output depths per core
D_IN = DQ + 8     # slab rows incl. halo
WPAD = 144        # 5 zeros | 128 data | 11 zeros
WOFF = 5
NCHUNK = 2
DC_OUT = DQ // NCHUNK          # 16
DC_IN = DC_OUT + 8             # 24
DCPAD = 30                     # 5 zeros | 24 data | 1 zero
DPOFF = 5
FLOW_D = DQ + 1                # 33
WIN3 = 729.0

N_IN = D_IN * WPAD             # 5760
N_CHUNK_IN = DC_IN * WPAD      # 3456
N_CHUNK_HB = DC_IN * W         # 3072  (H-boxed compact, per chunk)
N_DPAD = W * DCPAD             # 4096
N_BOX = W * DC_OUT             # 2048
N_RECON = DQ * W               # 4096
N_FLOW_C = FLOW_D * W          # 4224

# Use the fast fp32 path on the PE for the band matmuls (float32r).
USE_F32R = os.environ.get("GVSL_F32R", "0") == "1"

# acc_all columns
COL_CC0 = 0          # cc sum, chunk 0 / chunk 1
COL_MSE = 2
COL_DX = 3           # +c, W-axis diffs (3 channels)
COL_DY = 6           # +c, H-axis diffs
COL_DZ = 9           # +c, D-axis diffs

_CACHE = {}


def _build_program():
    nc = bacc.Bacc("TRN2", target_bir_lowering=False, debug=False, num_devices=8)

    d_imgsA = nc.dram_tensor("imgsA", [HP, N_IN], F32, kind="ExternalInput").ap()
    d_warped = nc.dram_tensor("warped", [HP, N_IN], F32, kind="ExternalInput").ap()
    d_recon = nc.dram_tensor("recon", [HP, N_RECON], F32, kind="ExternalInput").ap()
    d_flow = nc.dram_tensor("flow", [HP, 3 * N_FLOW_C], F32, kind="ExternalInput").ap()
    d_bandp = nc.dram_tensor("bandp", [HP, HP], F32, kind="ExternalInput").ap()
    d_bandn = nc.dram_tensor("bandn", [HP, HP], F32, kind="ExternalInput").ap()
    d_bidiag = nc.dram_tensor("bidiag", [HP, HP - 1], F32, kind="ExternalInput").ap()
    d_out = nc.dram_tensor("out", [HP, 16], F32, kind="ExternalOutput").ap()

    with tile.TileContext(nc) as tc:
        with tc.tile_pool(name="persist", bufs=1) as pp:
            acc = pp.tile([HP, 16], F32, tag="acc", name="acc")[:]
            eps_ap = pp.tile([HP, 1], F32, tag="epsc", name="epsc")[:]
            nc.gpsimd.memset(eps_ap, 1e-5)
            bandp = pp.tile([HP, HP], F32, tag="bandp", name="bandp")[:]
            bandn = pp.tile([HP, HP], F32, tag="bandn", name="bandn")[:]
            bidiag = pp.tile([HP, HP - 1], F32, tag="bidiag", name="bidiag")[:]
            inJ = pp.tile([HP, N_IN], F32, tag="inJ", name="inJ")[:]
            nc.sync.dma_start(out=bandp, in_=d_bandp)
            nc.sync.dma_start(out=bandn, in_=d_bandn)
            nc.sync.dma_start(out=bidiag, in_=d_bidiag)
            nc.sync.dma_start(out=inJ, in_=d_imgsA)
            inJ_r = inJ.rearrange("p (d w) -> p d w", w=WPAD)

            # ---------------- phase 1: smoothness + MSE ----------------
            with (
                tc.tile_pool(name="flowp", bufs=2) as fp,
                tc.tile_pool(name="diffp", bufs=2) as fdp,
                tc.tile_pool(name="reconp", bufs=1) as rp,
                tc.tile_pool(name="fpsum", bufs=4, space="PSUM") as fpsp,
            ):
                recon = rp.tile([HP, N_RECON], F32, tag="recon", name="recon")[:]
                nc.sync.dma_start(out=recon, in_=d_recon)
                recon_r = recon.rearrange("p (d w) -> p d w", w=W)

                # MSE = sum((imgsA - recon)^2) over the interior slab
                a_int = inJ_r[:, 4 : 4 + DQ, WOFF : WOFF + W]
                mbuf = fdp.tile([HP, N_RECON], F32, tag="dbuf", name="dbuf")[:]
                mbuf_r = mbuf.rearrange("p (d w) -> p d w", w=W)
                nc.vector.tensor_sub(mbuf_r, a_int, recon_r)
                nc.scalar.activation(
                    mbuf, mbuf, AF.Square, accum_out=acc[:, COL_MSE : COL_MSE + 1]
                )

                d_flow_r = d_flow.rearrange("p (c d w) -> p c d w", c=3, w=W)
                for c in range(3):
                    fc = fp.tile([HP, N_FLOW_C], F32, tag="fc", name="fc")[:]
                    nc.sync.dma_start(out=fc, in_=d_flow_r[:, c].rearrange("p d w -> p (d w)"))
                    fc_r = fc.rearrange("p (d w) -> p d w", w=W)

                    # W-axis diffs (innermost)
                    db = fdp.tile([HP, N_RECON], F32, tag="dbuf", name="dbuf")[:]
                    db_x = db.rearrange("p (d w) -> p d w", w=W)[:, :, 0 : W - 1]
                    nc.vector.tensor_sub(
                        db_x, fc_r[:, 0:DQ, 1:W], fc_r[:, 0:DQ, 0 : W - 1]
                    )
                    col = COL_DX + c
                    nc.scalar.activation(
                        db.rearrange("p (d w) -> p d w", w=W)[:, :, 0 : W - 1],
                        db.rearrange("p (d w) -> p d w", w=W)[:, :, 0 : W - 1],
                        AF.Square,
                        accum_out=acc[:, col : col + 1],
                    )

                    # H-axis diffs on the PE: psum = bidiag^T @ fc
                    #   out[m, :] = fc[m+1, :] - fc[m, :]
                    db = fdp.tile([HP, N_RECON], F32, tag="dbuf", name="dbuf")[:]
                    fc_flat = fc_r[:, 0:DQ, :].rearrange("p d w -> p (d w)")
                    for j in range(N_RECON // 512):
                        ps = fpsp.tile([HP, 512], F32, tag="fps", name="fps")[:]
                        nc.tensor.matmul(
                            ps[0 : HP - 1, :],
                            bidiag,
                            fc_flat[:, 512 * j : 512 * (j + 1)],
                            start=True,
                            stop=True,
                        )
                        nc.scalar.copy(
                            db[0 : HP - 1, 512 * j : 512 * (j + 1)], ps[0 : HP - 1, :]
                        )
                    col = COL_DY + c
                    nc.scalar.activation(
                        db[0 : HP - 1, :],
                        db[0 : HP - 1, :],
                        AF.Square,
                        accum_out=acc[0 : HP - 1, col : col + 1],
                    )

                    # D-axis diffs
                    db = fdp.tile([HP, N_RECON], F32, tag="dbuf", name="dbuf")[:]
                    nc.vector.tensor_sub(
                        db,
                        fc_r[:, 1 : DQ + 1, :].rearrange("p d w -> p (d w)"),
                        fc_r[:, 0:DQ, :].rearrange("p d w -> p (d w)"),
                    )
                    col = COL_DZ + c
                    nc.scalar.activation(
                        db, db, AF.Square, accum_out=acc[:, col : col + 1]
                    )

            # ---------------- phase 2: NCC ----------------
            with (
                tc.tile_pool(name="srcI", bufs=1) as sip,
                tc.tile_pool(name="prodp", bufs=2) as prp,
                tc.tile_pool(name="cump", bufs=2) as cup,
                tc.tile_pool(name="dpadp", bufs=2) as dpp,
                tc.tile_pool(name="boxp", bufs=1) as bxp,
                tc.tile_pool(name="scrp", bufs=1) as scp,
                tc.tile_pool(name="psum", bufs=6, space="PSUM") as psp,
            ):
                inI = sip.tile([HP, N_IN], F32, tag="inI", name="inI")[:]
                nc.sync.dma_start(out=inI, in_=d_warped)
                inI_r = inI.rearrange("p (d w) -> p d w", w=WPAD)

                if USE_F32R:
                    F32R = mybir.dt.float32r
                    bandp_mm = bandp.bitcast(F32R)
                    bandn_mm = bandn.bitcast(F32R)
                else:
                    bandp_mm = bandp
                    bandn_mm = bandn

                for ch in range(NCHUNK):
                    r0 = ch * DC_OUT  # first slab row of this chunk
                    Jc = inJ_r[:, r0 : r0 + DC_IN, :]
                    Ic = inI_r[:, r0 : r0 + DC_IN, :]
                    Jc2 = Jc.rearrange("p d w -> p (d w)")
                    Ic2 = Ic.rearrange("p d w -> p (d w)")

                    boxes = {}
                    for v in ("J", "I", "II", "JJ", "IJ"):
                        # source tensor for this volume
                        if v == "J":
                            src2 = Jc2
                        elif v == "I":
                            src2 = Ic2
                        else:
                            prod = prp.tile([HP, N_CHUNK_IN], F32, tag="prod", name="prod")[:]
                            if v == "II":
                                nc.scalar.activation(prod, Ic2, AF.Square)
                            elif v == "JJ":
                                nc.scalar.activation(prod, Jc2, AF.Square)
                            else:
                                nc.vector.tensor_mul(prod, Ic2, Jc2)
                            src2 = prod

                        # W-axis cumulative sum (box diff is fused into the
                        # H-box matmul below via the +/- band pair)
                        cum = cup.tile([HP, N_DPAD], F32, tag="cum", name="cum")[:]
                        nc.vector.tensor_tensor_scan(
                            cum[:, 0:N_CHUNK_IN],
                            src2,
                            src2,
                            0.0,
                            op0=ALU.add,
                            op1=ALU.bypass,
                        )
                        cum_r = cum[:, 0:N_CHUNK_IN].rearrange(
                            "p (d w) -> p d w", w=WPAD
                        )

                        # dpad: [p, (w, dp)] with dp innermost, zero pads
                        dpad = dpp.tile([HP, N_DPAD], F32, tag="dpad", name="dpad")[:]
                        dpad_r = dpad.rearrange("p (w dp) -> p w dp", dp=DCPAD)
                        nc.gpsimd.memset(dpad_r[:, :, 0:DPOFF], 0.0)
                        nc.gpsimd.memset(dpad_r[:, :, DPOFF + DC_IN : DCPAD], 0.0)

                        # H-box matmuls with fused W-box difference:
                        #   psum = band^T @ cum[.., w+9] - band^T @ cum[.., w+0]
                        for j in range(N_CHUNK_HB // 512):
                            dlo = 4 * j
                            ps = psp.tile([HP, 512], F32, tag="ps", name="ps")[:]
                            rhs9 = cum_r[:, dlo : dlo + 4, 9 : 9 + W]
                            rhs0 = cum_r[:, dlo : dlo + 4, 0:W]
                            if USE_F32R:
                                rhs9 = rhs9.bitcast(mybir.dt.float32r)
                                rhs0 = rhs0.bitcast(mybir.dt.float32r)
                            nc.tensor.matmul(
                                ps, bandp_mm, rhs9, start=True, stop=False
                            )
                            nc.tensor.matmul(
                                ps, bandn_mm, rhs0, start=False, stop=True
                            )
                            # evacuate into dpad, transposed to (w, dp)
                            ps_wd = ps.rearrange("p (s w) -> p w s", w=W)
                            nc.scalar.copy(
                                dpad_r[:, :, DPOFF + dlo : DPOFF + dlo + 4], ps_wd
                            )

                        # D-axis cumsum + diff -> final 9^3 box sums
                        cumd = cup.tile([HP, N_DPAD], F32, tag="cum", name="cum")[:]
                        nc.vector.tensor_tensor_scan(
                            cumd, dpad, dpad, 0.0, op0=ALU.add, op1=ALU.bypass
                        )
                        cumd_r = cumd.rearrange("p (w dp) -> p w dp", dp=DCPAD)
                        B = bxp.tile([HP, N_BOX], F32, tag=f"box{v}", name=f"box{v}")[:]
                        B_r = B.rearrange("p (w d) -> p w d", d=DC_OUT)
                        nc.vector.tensor_sub(
                            B_r,
                            cumd_r[:, :, 13 : 13 + DC_OUT],
                            cumd_r[:, :, 4 : 4 + DC_OUT],
                        )
                        boxes[v] = B

                    # ---- cc math on [128, 2048] box sums ----
                    BJ, BI = boxes["J"], boxes["I"]
                    BII, BJJ, BIJ = boxes["II"], boxes["JJ"], boxes["IJ"]
                    s1 = scp.tile([HP, N_BOX], F32, tag="s1", name="s1")[:]
                    s2 = scp.tile([HP, N_BOX], F32, tag="s2", name="s2")[:]
                    s3 = scp.tile([HP, N_BOX], F32, tag="s3", name="s3")[:]

                    nc.vector.tensor_mul(s1, BI, BJ)
                    nc.vector.scalar_tensor_tensor(
                        s2, s1, -1.0 / WIN3, BIJ, op0=ALU.mult, op1=ALU.add
                    )  # cross
                    nc.scalar.activation(s1, s2, AF.Square)       # cross^2
                    nc.scalar.activation(s2, s1, AF.Ln)           # ln(cross^2)
                    nc.scalar.activation(s1, BI, AF.Square)
                    nc.vector.scalar_tensor_tensor(
                        s3, s1, -1.0 / WIN3, BII, op0=ALU.mult, op1=ALU.add
                    )  # I_var
                    nc.scalar.activation(s1, BJ, AF.Square)
                    nc.vector.scalar_tensor_tensor(
                        BII, s1, -1.0 / WIN3, BJJ, op0=ALU.mult, op1=ALU.add
                    )  # J_var (overwrites dead BII)
                    nc.vector.tensor_mul(s1, s3, BII)             # I_var * J_var
                    nc.scalar.activation(s3, s1, AF.Ln, bias=eps_ap)
                    nc.vector.tensor_sub(s1, s2, s3)
                    col = COL_CC0 + ch
                    nc.scalar.activation(
                        s3, s1, AF.Exp, accum_out=acc[:, col : col + 1]
                    )

            nc.sync.dma_start(out=d_out, in_=acc)

    nc.compile()
    return nc


def _make_band() -> tuple[np.ndarray, np.ndarray, np.ndarray]:
    k = np.arange(HP)
    band = (np.abs(k[:, None] - k[None, :]) <= 4).astype(np.float32)
    m = np.arange(HP - 1)
    bidiag = np.zeros((HP, HP - 1), np.float32)
    bidiag[m + 1, m] = 1.0
    bidiag[m, m] = -1.0
    return band, -band, bidiag


def _shard_inputs(imgsA, recon_A, warped_BA, flow_BA):
    bandp, bandn, bidiag = _make_band()
    in_maps = []
    for core in range(8):
        b, q = divmod(core, 4)
        d0 = DQ * q

        def slab(vol):
            s = np.zeros((HP, D_IN, WPAD), np.float32)
            lo, hi = d0 - 4, d0 + DQ + 4
            clo, chi = max(lo, 0), min(hi, D_FULL)
            s[:, clo - lo : chi - lo, WOFF : WOFF + W] = np.ascontiguousarray(
                vol[clo:chi].transpose(1, 0, 2)
            )
            return s.reshape(HP, N_IN)

        rec = np.ascontiguousarray(
            recon_A[b, 0, d0 : d0 + DQ].transpose(1, 0, 2)
        ).reshape(HP, N_RECON)

        fl = np.empty((HP, 3, FLOW_D, W), np.float32)
        hi = min(d0 + FLOW_D, D_FULL)
        n = hi - d0
        fl[:, :, :n] = flow_BA[b, :, d0:hi].transpose(2, 0, 1, 3)
        if n < FLOW_D:
            fl[:, :, n:] = fl[:, :, n - 1 : n]

        in_maps.append(
            {
                "imgsA": slab(imgsA[b, 0]),
                "warped": slab(warped_BA[b, 0]),
                "recon": rec,
                "flow": np.ascontiguousarray(fl).reshape(HP, 3 * N_FLOW_C),
                "bandp": bandp,
                "bandn": bandn,
                "bidiag": bidiag,
            }
        )
    return in_maps


def _install_profile_shim():
    """Wire up NTFF profiling under axon when antenv.axon_hooks is absent."""
    try:
        import antenv.axon_hooks  # noqa: F401

        return True
    except ImportError:
        pass
    import contextlib
    import ctypes
    import types

    so_path = "/opt/axon/libaxon_pjrt.so"
    if not os.path.exists(so_path):
        return False
    lib = ctypes.CDLL(so_path)
    if not hasattr(lib, "axon_start_nrt_profile"):
        return False
    lib.axon_start_nrt_profile.argtypes = [
        ctypes.POINTER(ctypes.c_int64),
        ctypes.c_size_t,
    ]
    lib.axon_start_nrt_profile.restype = ctypes.c_int64
    lib.axon_stop_nrt_profile.argtypes = [ctypes.c_char_p]
    lib.axon_stop_nrt_profile.restype = ctypes.c_int64

    @contextlib.contextmanager
    def _hook(output_dir, device_ids):
        import jax

        jax.devices()
        if device_ids:
            ids = (ctypes.c_int64 * len(device_ids))(*device_ids)
            rc = lib.axon_start_nrt_profile(ids, len(device_ids))
        else:
            rc = lib.axon_start_nrt_profile(None, 0)
        if rc != 0:
            raise RuntimeError(f"axon_start_nrt_profile rc={rc}")
        try:
            yield
        finally:
            n = lib.axon_stop_nrt_profile(str(output_dir).encode())
            print(f"ntff profile: {n} file(s) written to {output_dir}")

    mod = types.ModuleType("antenv.axon_hooks")
    mod.get_axon_ntff_profile_hook = lambda: _hook
    mod.set_axon_ntff_profile_hook = lambda h: None
    import antenv

    sys.modules["antenv.axon_hooks"] = mod
    antenv.axon_hooks = mod

    # keep profile artifacts local instead of uploading to fishnet
    import concourse.bass_utils as _bu

    _bu.upload_artifacts = lambda tmpdir: tmpdir
    return True


LAST_EXEC_NS = None
LAST_RESULTS = None


def kernel(imgsA, recon_A, warped_BA, flow_BA):
    global LAST_EXEC_NS, LAST_RESULTS
    if "nc" not in _CACHE:
        _CACHE["nc"] = _build_program()
    nc = _CACHE["nc"]

    in_maps = _shard_inputs(
        np.asarray(imgsA, np.float32),
        np.asarray(recon_A, np.float32),
        np.asarray(warped_BA, np.float32),
        np.asarray(flow_BA, np.float32),
    )
    trace = os.environ.get("GVSL_TRACE", "0") == "1"
    if trace:
        trace = _install_profile_shim()
    tmpdir = os.environ.get("GVSL_TRACE_DIR") or None
    res = run_bass_kernel_spmd(
        nc, in_maps, core_ids=list(range(8)), trace=trace, tmpdir=tmpdir
    )
    LAST_EXEC_NS = res.exec_time_ns
    LAST_RESULTS = res

    cc = mse = dx = dy = dz = 0.0
    for r in res.results:
        o = np.asarray(r["out"], np.float64)
        cc += o[:, COL_CC0].sum() + o[:, COL_CC0 + 1].sum()
        mse += o[:, COL_MSE].sum()
        dx += o[:, COL_DX : COL_DX + 3].sum()
        dy += o[: HP - 1, COL_DY : COL_DY + 3].sum()
        dz += o[:, COL_DZ : COL_DZ + 3].sum()

    n_vox = 2 * 1 * 128 * 128 * 128
    n_d = 2 * 3 * 127 * 128 * 128
    ncc_loss = 1.0 - cc / n_vox
    mse_loss = mse / n_vox
    smooth_loss = (dx / n_d + dy / n_d + dz / n_d) / 3.0
    return (
        np.float32(ncc_loss),
        np.float32(mse_loss),
        np.float32(smooth_loss),
    )


# revision 22
# speedup vs baseline: 1.9588x; 1.2868x over previous
"""GVSL loss (NCC + MSE + smoothness) as a distributed Bass kernel on 8 TRN2 cores.

Sharding: batch(2) x depth-quarters(4) = 8 shards. Each core computes partial
sums for its 32-deep output slab (with 4-voxel halo for the 9^3 box filter);
the final scalar reductions happen on the host.
"""

import os
import sys

for _p in ("/opt/trn_rl_repo",):
    if _p not in sys.path:
        sys.path.insert(0, _p)

import numpy as np

import concourse.bass as bass
import concourse.tile as tile
from concourse import bacc, mybir
from concourse.bass_utils import run_bass_kernel_spmd

F32 = mybir.dt.float32
AF = mybir.ActivationFunctionType
ALU = mybir.AluOpType

HP = 128          # partitions (H axis)
W = 128
D_FULL = 128
DQ = 32           # output depths per core
D_IN = DQ + 8     # slab rows incl. halo
WPAD = 144        # 5 zeros | 128 data | 11 zeros
WOFF = 5
NCHUNK = 2
DC_OUT = DQ // NCHUNK          # 16
DC_IN = DC_OUT + 8             # 24
DCPAD = 30                     # 5 zeros | 24 data | 1 zero
DPOFF = 5
FLOW_D = DQ + 1                # 33
WIN3 = 729.0

N_IN = D_IN * WPAD             # 5760
N_CHUNK_IN = DC_IN * WPAD      # 3456
N_CHUNK_HB = DC_IN * W         # 3072  (H-boxed compact, per chunk)
N_DPAD = W * DCPAD             # 4096
N_BOX = W * DC_OUT             # 2048
N_RECON = DQ * W               # 4096
N_FLOW_C = FLOW_D * W          # 4224

# Use the fast fp32 path on the PE for the band matmuls (float32r).
USE_F32R = os.environ.get("GVSL_F32R", "0") == "1"

# acc_all columns
COL_CC0 = 0          # cc sum, chunk 0 / chunk 1
COL_MSE = 2
COL_DX = 3           # +c, W-axis diffs (3 channels)
COL_DY = 6           # +c, H-axis diffs
COL_DZ = 9           # +c, D-axis diffs

_CACHE = {}


def _build_program():
    nc = bacc.Bacc("TRN2", target_bir_lowering=False, debug=False, num_devices=8)

    d_imgsA = nc.dram_tensor("imgsA", [HP, N_IN], F32, kind="ExternalInput").ap()
    d_warped = nc.dram_tensor("warped", [HP, N_IN], F32, kind="ExternalInput").ap()
    d_recon = nc.dram_tensor("recon", [HP, N_RECON], F32, kind="ExternalInput").ap()
    d_flow = nc.dram_tensor("flow", [HP, 3 * N_FLOW_C], F32, kind="ExternalInput").ap()
    d_bandp = nc.dram_tensor("bandp", [HP, HP], F32, kind="ExternalInput").ap()
    d_bandn = nc.dram_tensor("bandn", [HP, HP], F32, kind="ExternalInput").ap()
    d_bidiag = nc.dram_tensor("bidiag", [HP, HP - 1], F32, kind="ExternalInput").ap()
    d_out = nc.dram_tensor("out", [HP, 16], F32, kind="ExternalOutput").ap()

    with tile.TileContext(nc) as tc:
        with tc.tile_pool(name="persist", bufs=1) as pp:
            acc = pp.tile([HP, 16], F32, tag="acc", name="acc")[:]
            eps_ap = pp.tile([HP, 1], F32, tag="epsc", name="epsc")[:]
            nc.gpsimd.memset(eps_ap, 1e-5)
            bandp = pp.tile([HP, HP], F32, tag="bandp", name="bandp")[:]
            bandn = pp.tile([HP, HP], F32, tag="bandn", name="bandn")[:]
            bidiag = pp.tile([HP, HP - 1], F32, tag="bidiag", name="bidiag")[:]
            inJ = pp.tile([HP, N_IN], F32, tag="inJ", name="inJ")[:]
            nc.sync.dma_start(out=bandp, in_=d_bandp)
            nc.sync.dma_start(out=bandn, in_=d_bandn)
            nc.sync.dma_start(out=bidiag, in_=d_bidiag)
            nc.sync.dma_start(out=inJ, in_=d_imgsA)
            inJ_r = inJ.rearrange("p (d w) -> p d w", w=WPAD)

            # ---------------- phase 1: smoothness + MSE ----------------
            with (
                tc.tile_pool(name="flowp", bufs=2) as fp,
                tc.tile_pool(name="diffp", bufs=2) as fdp,
                tc.tile_pool(name="reconp", bufs=1) as rp,
                tc.tile_pool(name="fpsum", bufs=4, space="PSUM") as fpsp,
            ):
                recon = rp.tile([HP, N_RECON], F32, tag="recon", name="recon")[:]
                nc.sync.dma_start(out=recon, in_=d_recon)
                recon_r = recon.rearrange("p (d w) -> p d w", w=W)

                # MSE = sum((imgsA - recon)^2) over the interior slab
                a_int = inJ_r[:, 4 : 4 + DQ, WOFF : WOFF + W]
                mbuf = fdp.tile([HP, N_RECON], F32, tag="dbuf", name="dbuf")[:]
                mbuf_r = mbuf.rearrange("p (d w) -> p d w", w=W)
                nc.vector.tensor_sub(mbuf_r, a_int, recon_r)
                nc.scalar.activation(
                    mbuf, mbuf, AF.Square, accum_out=acc[:, COL_MSE : COL_MSE + 1]
                )

                d_flow_r = d_flow.rearrange("p (c d w) -> p c d w", c=3, w=W)
                for c in range(3):
                    fc = fp.tile([HP, N_FLOW_C], F32, tag="fc", name="fc")[:]
                    nc.sync.dma_start(out=fc, in_=d_flow_r[:, c].rearrange("p d w -> p (d w)"))
                    fc_r = fc.rearrange("p (d w) -> p d w", w=W)

                    # W-axis diffs (innermost)
                    db = fdp.tile([HP, N_RECON], F32, tag="dbuf", name="dbuf")[:]
                    db_x = db.rearrange("p (d w) -> p d w", w=W)[:, :, 0 : W - 1]
                    nc.vector.tensor_sub(
                        db_x, fc_r[:, 0:DQ, 1:W], fc_r[:, 0:DQ, 0 : W - 1]
                    )
                    col = COL_DX + c
                    nc.scalar.activation(
                        db.rearrange("p (d w) -> p d w", w=W)[:, :, 0 : W - 1],
                        db.rearrange("p (d w) -> p d w", w=W)[:, :, 0 : W - 1],
                        AF.Square,
                        accum_out=acc[:, col : col + 1],
                    )

                    # H-axis diffs on the PE: psum = bidiag^T @ fc
                    #   out[m, :] = fc[m+1, :] - fc[m, :]
                    db = fdp.tile([HP, N_RECON], F32, tag="dbuf", name="dbuf")[:]
                    fc_flat = fc_r[:, 0:DQ, :].rearrange("p d w -> p (d w)")
                    for j in range(N_RECON // 512):
                        ps = fpsp.tile([HP, 512], F32, tag="fps", name="fps")[:]
                        nc.tensor.matmul(
                            ps[0 : HP - 1, :],
                            bidiag,
                            fc_flat[:, 512 * j : 512 * (j + 1)],
                            start=True,
                            stop=True,
                        )
                        nc.scalar.copy(
                            db[0 : HP - 1, 512 * j : 512 * (j + 1)], ps[0 : HP - 1, :]
                        )
                    col = COL_DY + c
                    nc.scalar.activation(
                        db[0 : HP - 1, :],
                        db[0 : HP - 1, :],
                        AF.Square,
                        accum_out=acc[0 : HP - 1, col : col + 1],
                    )

                    # D-axis diffs
                    db = fdp.tile([HP, N_RECON], F32, tag="dbuf", name="dbuf")[:]
                    nc.vector.tensor_sub(
                        db,
                        fc_r[:, 1 : DQ + 1, :].rearrange("p d w -> p (d w)"),
                        fc_r[:, 0:DQ, :].rearrange("p d w -> p (d w)"),
                    )
                    col = COL_DZ + c
                    nc.scalar.activation(
                        db, db, AF.Square, accum_out=acc[:, col : col + 1]
                    )

            # ---------------- phase 2: NCC ----------------
            with (
                tc.tile_pool(name="srcI", bufs=1) as sip,
                tc.tile_pool(name="prodp", bufs=2) as prp,
                tc.tile_pool(name="cump", bufs=2) as cup,
                tc.tile_pool(name="dpadp", bufs=2) as dpp,
                tc.tile_pool(name="boxp", bufs=1) as bxp,
                tc.tile_pool(name="scrp", bufs=1) as scp,
                tc.tile_pool(name="psum", bufs=6, space="PSUM") as psp,
            ):
                inI = sip.tile([HP, N_IN], F32, tag="inI", name="inI")[:]
                nc.sync.dma_start(out=inI, in_=d_warped)
                inI_r = inI.rearrange("p (d w) -> p d w", w=WPAD)

                if USE_F32R:
                    F32R = mybir.dt.float32r
                    bandp_mm = bandp.bitcast(F32R)
                    bandn_mm = bandn.bitcast(F32R)
                else:
                    bandp_mm = bandp
                    bandn_mm = bandn

                for ch in range(NCHUNK):
                    r0 = ch * DC_OUT  # first slab row of this chunk
                    Jc = inJ_r[:, r0 : r0 + DC_IN, :]
                    Ic = inI_r[:, r0 : r0 + DC_IN, :]
                    Jc2 = Jc.rearrange("p d w -> p (d w)")
                    Ic2 = Ic.rearrange("p d w -> p (d w)")

                    boxes = {}
                    for v in ("J", "I", "II", "JJ", "IJ"):
                        # source tensor for this volume
                        if v == "J":
                            src2 = Jc2
                        elif v == "I":
                            src2 = Ic2
                        else:
                            prod = prp.tile([HP, N_CHUNK_IN], F32, tag="prod", name="prod")[:]
                            if v == "II":
                                nc.scalar.activation(prod, Ic2, AF.Square)
                            elif v == "JJ":
                                nc.scalar.activation(prod, Jc2, AF.Square)
                            else:
                                nc.vector.tensor_mul(prod, Ic2, Jc2)
                            src2 = prod

                        # W-axis cumulative sum, split into two chained halves
                        # so the H-box matmuls can start after the first half
                        # (box diff is fused into the matmul via the +/- bands)
                        NH = N_CHUNK_IN // 2          # 1728 = 12 d-rows
                        cum_a = cup.tile([HP, NH], F32, tag="cuma", name="cuma")[:]
                        cum_b = cup.tile([HP, NH], F32, tag="cumb", name="cumb")[:]
                        nc.vector.tensor_tensor_scan(
                            cum_a, src2[:, 0:NH], src2[:, 0:NH],
                            0.0, op0=ALU.add, op1=ALU.bypass,
                        )
                        nc.vector.tensor_tensor_scan(
                            cum_b, src2[:, NH:], src2[:, NH:],
                            cum_a[:, NH - 1 : NH], op0=ALU.add, op1=ALU.bypass,
                        )
                        cum_ar = cum_a.rearrange("p (d w) -> p d w", w=WPAD)
                        cum_br = cum_b.rearrange("p (d w) -> p d w", w=WPAD)

                        # dpad: [p, (w, dp)] with dp innermost, zero pads
                        dpad = dpp.tile([HP, N_DPAD], F32, tag="dpad", name="dpad")[:]
                        dpad_r = dpad.rearrange("p (w dp) -> p w dp", dp=DCPAD)
                        nc.gpsimd.memset(dpad_r[:, :, 0:DPOFF], 0.0)
                        nc.gpsimd.memset(dpad_r[:, :, DPOFF + DC_IN : DCPAD], 0.0)

                        # H-box matmuls with fused W-box difference:
                        #   psum = band^T @ cum[.., w+9] - band^T @ cum[.., w+0]
                        for j in range(N_CHUNK_HB // 512):
                            dlo = 4 * j
                            ps = psp.tile([HP, 512], F32, tag="ps", name="ps")[:]
                            cr = cum_ar if j < 3 else cum_br
                            dl = dlo if j < 3 else dlo - 12
                            rhs9 = cr[:, dl : dl + 4, 9 : 9 + W]
                            rhs0 = cr[:, dl : dl + 4, 0:W]
                            if USE_F32R:
                                rhs9 = rhs9.bitcast(mybir.dt.float32r)
                                rhs0 = rhs0.bitcast(mybir.dt.float32r)
                            nc.tensor.matmul(
                                ps, bandp_mm, rhs9, start=True, stop=False
                            )
                            nc.tensor.matmul(
                                ps, bandn_mm, rhs0, start=False, stop=True
                            )
                            # evacuate into dpad, transposed to (w, dp)
                            ps_wd = ps.rearrange("p (s w) -> p w s", w=W)
                            nc.scalar.copy(
                                dpad_r[:, :, DPOFF + dlo : DPOFF + dlo + 4], ps_wd
                            )

                        # D-axis cumsum + diff -> final 9^3 box sums
                        nc.vector.tensor_tensor_scan(
                            dpad, dpad, dpad, 0.0, op0=ALU.add, op1=ALU.bypass
                        )
                        cumd_r = dpad.rearrange("p (w dp) -> p w dp", dp=DCPAD)
                        B = bxp.tile([HP, N_BOX], F32, tag=f"box{v}", name=f"box{v}")[:]
                        B_r = B.rearrange("p (w d) -> p w d", d=DC_OUT)
                        nc.vector.tensor_sub(
                            B_r,
                            cumd_r[:, :, 13 : 13 + DC_OUT],
                            cumd_r[:, :, 4 : 4 + DC_OUT],
                        )
                        boxes[v] = B

                    # ---- cc math on [128, 2048] box sums ----
                    BJ, BI = boxes["J"], boxes["I"]
                    BII, BJJ, BIJ = boxes["II"], boxes["JJ"], boxes["IJ"]
                    s1 = scp.tile([HP, N_BOX], F32, tag="s1", name="s1")[:]
                    s2 = scp.tile([HP, N_BOX], F32, tag="s2", name="s2")[:]
                    s3 = scp.tile([HP, N_BOX], F32, tag="s3", name="s3")[:]

                    nc.vector.tensor_mul(s1, BI, BJ)
                    nc.vector.scalar_tensor_tensor(
                        s2, s1, -1.0 / WIN3, BIJ, op0=ALU.mult, op1=ALU.add
                    )  # cross
                    nc.scalar.activation(s1, s2, AF.Square)       # cross^2
                    nc.scalar.activation(s2, s1, AF.Ln)           # ln(cross^2)
                    nc.scalar.activation(s1, BI, AF.Square)
                    nc.vector.scalar_tensor_tensor(
                        s3, s1, -1.0 / WIN3, BII, op0=ALU.mult, op1=ALU.add
                    )  # I_var
                    nc.scalar.activation(s1, BJ, AF.Square)
                    nc.vector.scalar_tensor_tensor(
                        BII, s1, -1.0 / WIN3, BJJ, op0=ALU.mult, op1=ALU.add
                    )  # J_var (overwrites dead BII)
                    nc.vector.tensor_mul(s1, s3, BII)             # I_var * J_var
                    nc.scalar.activation(s3, s1, AF.Ln, bias=eps_ap)
                    nc.vector.tensor_sub(s1, s2, s3)
                    col = COL_CC0 + ch
                    nc.scalar.activation(
                        s3, s1, AF.Exp, accum_out=acc[:, col : col + 1]
                    )

            nc.sync.dma_start(out=d_out, in_=acc)

    nc.compile()
    return nc


def _make_band() -> tuple[np.ndarray, np.ndarray, np.ndarray]:
    k = np.arange(HP)
    band = (np.abs(k[:, None] - k[None, :]) <= 4).astype(np.float32)
    m = np.arange(HP - 1)
    bidiag = np.zeros((HP, HP - 1), np.float32)
    bidiag[m + 1, m] = 1.0
    bidiag[m, m] = -1.0
    return band, -band, bidiag


def _shard_inputs(imgsA, recon_A, warped_BA, flow_BA):
    bandp, bandn, bidiag = _make_band()
    in_maps = []
    for core in range(8):
        b, q = divmod(core, 4)
        d0 = DQ * q

        def slab(vol):
            s = np.zeros((HP, D_IN, WPAD), np.float32)
            lo, hi = d0 - 4, d0 + DQ + 4
            clo, chi = max(lo, 0), min(hi, D_FULL)
            s[:, clo - lo : chi - lo, WOFF : WOFF + W] = np.ascontiguousarray(
                vol[clo:chi].transpose(1, 0, 2)
            )
            return s.reshape(HP, N_IN)

        rec = np.ascontiguousarray(
            recon_A[b, 0, d0 : d0 + DQ].transpose(1, 0, 2)
        ).reshape(HP, N_RECON)

        fl = np.empty((HP, 3, FLOW_D, W), np.float32)
        hi = min(d0 + FLOW_D, D_FULL)
        n = hi - d0
        fl[:, :, :n] = flow_BA[b, :, d0:hi].transpose(2, 0, 1, 3)
        if n < FLOW_D:
            fl[:, :, n:] = fl[:, :, n - 1 : n]

        in_maps.append(
            {
                "imgsA": slab(imgsA[b, 0]),
                "warped": slab(warped_BA[b, 0]),
                "recon": rec,
                "flow": np.ascontiguousarray(fl).reshape(HP, 3 * N_FLOW_C),
                "bandp": bandp,
                "bandn": bandn,
                "bidiag": bidiag,
            }
        )
    return in_maps


def _install_profile_shim():
    """Wire up NTFF profiling under axon when antenv.axon_hooks is absent."""
    try:
        import antenv.axon_hooks  # noqa: F401

        return True
    except ImportError:
        pass
    import contextlib
    import ctypes
    import types

    so_path = "/opt/axon/libaxon_pjrt.so"
    if not os.path.exists(so_path):
        return False
    lib = ctypes.CDLL(so_path)
    if not hasattr(lib, "axon_start_nrt_profile"):
        return False
    lib.axon_start_nrt_profile.argtypes = [
        ctypes.POINTER(ctypes.c_int64),
        ctypes.c_size_t,
    ]
    lib.axon_start_nrt_profile.restype = ctypes.c_int64
    lib.axon_stop_nrt_profile.argtypes = [ctypes.c_char_p]
    lib.axon_stop_nrt_profile.restype = ctypes.c_int64

    @contextlib.contextmanager
    def _hook(output_dir, device_ids):
        import jax

        jax.devices()
        if device_ids:
            ids = (ctypes.c_int64 * len(device_ids))(*device_ids)
            rc = lib.axon_start_nrt_profile(ids, len(device_ids))
        else:
            rc = lib.axon_start_nrt_profile(None, 0)
        if rc != 0:
            raise RuntimeError(f"axon_start_nrt_profile rc={rc}")
        try:
            yield
        finally:
            n = lib.axon_stop_nrt_profile(str(output_dir).encode())
            print(f"ntff profile: {n} file(s) written to {output_dir}")

    mod = types.ModuleType("antenv.axon_hooks")
    mod.get_axon_ntff_profile_hook = lambda: _hook
    mod.set_axon_ntff_profile_hook = lambda h: None
    import antenv

    sys.modules["antenv.axon_hooks"] = mod
    antenv.axon_hooks = mod

    # keep profile artifacts local instead of uploading to fishnet
    import concourse.bass_utils as _bu

    _bu.upload_artifacts = lambda tmpdir: tmpdir
    return True


LAST_EXEC_NS = None
LAST_RESULTS = None


def kernel(imgsA, recon_A, warped_BA, flow_BA):
    global LAST_EXEC_NS, LAST_RESULTS
    if "nc" not in _CACHE:
        _CACHE["nc"] = _build_program()
    nc = _CACHE["nc"]

    in_maps = _shard_inputs(
        np.asarray(imgsA, np.float32),
        np.asarray(recon_A, np.float32),
        np.asarray(warped_BA, np.float32),
        np.asarray(flow_BA, np.float32),
    )
    trace = os.environ.get("GVSL_TRACE", "0") == "1"
    if trace:
        trace = _install_profile_shim()
    tmpdir = os.environ.get("GVSL_TRACE_DIR") or None
    res = run_bass_kernel_spmd(
        nc, in_maps, core_ids=list(range(8)), trace=trace, tmpdir=tmpdir
    )
    LAST_EXEC_NS = res.exec_time_ns
    LAST_RESULTS = res

    cc = mse = dx = dy = dz = 0.0
    for r in res.results:
        o = np.asarray(r["out"], np.float64)
        cc += o[:, COL_CC0].sum() + o[:, COL_CC0 + 1].sum()
        mse += o[:, COL_MSE].sum()
        dx += o[:, COL_DX : COL_DX + 3].sum()
        dy += o[: HP - 1, COL_DY : COL_DY + 3].sum()
        dz += o[:, COL_DZ : COL_DZ + 3].sum()

    n_vox = 2 * 1 * 128 * 128 * 128
    n_d = 2 * 3 * 127 * 128 * 128
    ncc_loss = 1.0 - cc / n_vox
    mse_loss = mse / n_vox
    smooth_loss = (dx / n_d + dy / n_d + dz / n_d) / 3.0
    return (
        np.float32(ncc_loss),
        np.float32(mse_loss),
        np.float32(smooth_loss),
    )


# revision 24
# speedup vs baseline: 2.1204x; 1.0825x over previous
"""GVSL loss (NCC + MSE + smoothness) as a distributed Bass kernel on 8 TRN2 cores.

Sharding: batch(2) x depth-quarters(4) = 8 shards. Each core computes partial
sums for its 32-deep output slab (with 4-voxel halo for the 9^3 box filter);
the final scalar reductions happen on the host.
"""

import os
import sys

for _p in ("/opt/trn_rl_repo",):
    if _p not in sys.path:
        sys.path.insert(0, _p)

import numpy as np

import concourse.bass as bass
import concourse.tile as tile
from concourse import bacc, mybir
from concourse.bass_utils import run_bass_kernel_spmd

F32 = mybir.dt.float32
AF = mybir.ActivationFunctionType
ALU = mybir.AluOpType

HP = 128          # partitions (H axis)
W = 128
D_FULL = 128
DQ = 32           # output depths per core
D_IN = DQ + 8     # slab rows incl. halo
WPAD = 140        # 5 zeros | 128 data | 7 zeros
WOFF = 5
NCHUNK = 2
DC_OUT = DQ // NCHUNK          # 16
DC_IN = DC_OUT + 8             # 24
DCPAD = 26                     # 1 zero | 24 data | 1 zero
DPOFF = 1
FLOW_D = DQ + 1                # 33
WIN3 = 729.0

N_IN = D_IN * WPAD             # 5760
N_CHUNK_IN = DC_IN * WPAD      # 3456
N_CHUNK_HB = DC_IN * W         # 3072  (H-boxed compact, per chunk)
N_DPAD = W * DCPAD             # 4096
N_BOX = W * DC_OUT             # 2048
N_RECON = DQ * W               # 4096
N_FLOW_C = FLOW_D * W          # 4224

# Use the fast fp32 path on the PE for the band matmuls (float32r).
USE_F32R = os.environ.get("GVSL_F32R", "0") == "1"

# acc_all columns
COL_CC0 = 0          # cc sum, chunk 0 / chunk 1
COL_MSE = 2
COL_DX = 3           # +c, W-axis diffs (3 channels)
COL_DY = 6           # +c, H-axis diffs
COL_DZ = 9           # +c, D-axis diffs

_CACHE = {}


def _build_program():
    nc = bacc.Bacc("TRN2", target_bir_lowering=False, debug=False, num_devices=8)

    d_imgsA = nc.dram_tensor("imgsA", [HP, N_IN], F32, kind="ExternalInput").ap()
    d_warped = nc.dram_tensor("warped", [HP, N_IN], F32, kind="ExternalInput").ap()
    d_recon = nc.dram_tensor("recon", [HP, N_RECON], F32, kind="ExternalInput").ap()
    d_flow = nc.dram_tensor("flow", [HP, 3 * N_FLOW_C], F32, kind="ExternalInput").ap()
    d_bandp = nc.dram_tensor("bandp", [HP, HP], F32, kind="ExternalInput").ap()
    d_bandn = nc.dram_tensor("bandn", [HP, HP], F32, kind="ExternalInput").ap()
    d_bidiag = nc.dram_tensor("bidiag", [HP, HP - 1], F32, kind="ExternalInput").ap()
    d_out = nc.dram_tensor("out", [HP, 16], F32, kind="ExternalOutput").ap()

    with tile.TileContext(nc) as tc:
        with tc.tile_pool(name="persist", bufs=1) as pp:
            acc = pp.tile([HP, 16], F32, tag="acc", name="acc")[:]
            eps_ap = pp.tile([HP, 1], F32, tag="epsc", name="epsc")[:]
            nc.gpsimd.memset(eps_ap, 1e-5)
            bandp = pp.tile([HP, HP], F32, tag="bandp", name="bandp")[:]
            bandn = pp.tile([HP, HP], F32, tag="bandn", name="bandn")[:]
            bidiag = pp.tile([HP, HP - 1], F32, tag="bidiag", name="bidiag")[:]
            inJ = pp.tile([HP, N_IN], F32, tag="inJ", name="inJ")[:]
            nc.sync.dma_start(out=bandp, in_=d_bandp)
            nc.sync.dma_start(out=bandn, in_=d_bandn)
            nc.sync.dma_start(out=bidiag, in_=d_bidiag)
            nc.sync.dma_start(out=inJ, in_=d_imgsA)
            inJ_r = inJ.rearrange("p (d w) -> p d w", w=WPAD)

            # NCC front-end pools opened early so their space does not
            # overlap the flow pools: lets NCC scans overlap phase 1.
            from contextlib import ExitStack as _ES
            _es = _ES()
            sip = _es.enter_context(tc.tile_pool(name="srcI", bufs=1))
            prp = _es.enter_context(tc.tile_pool(name="prodp", bufs=2))
            cup = _es.enter_context(tc.tile_pool(name="cump", bufs=2))

            # ---------------- phase 1: smoothness + MSE ----------------
            with (
                tc.tile_pool(name="flowp", bufs=2) as fp,
                tc.tile_pool(name="diffp", bufs=2) as fdp,
                tc.tile_pool(name="reconp", bufs=1) as rp,
                tc.tile_pool(name="fpsum", bufs=4, space="PSUM") as fpsp,
            ):
                recon = rp.tile([HP, N_RECON], F32, tag="recon", name="recon")[:]
                nc.sync.dma_start(out=recon, in_=d_recon)
                recon_r = recon.rearrange("p (d w) -> p d w", w=W)

                # MSE = sum((imgsA - recon)^2) over the interior slab
                a_int = inJ_r[:, 4 : 4 + DQ, WOFF : WOFF + W]
                mbuf = fdp.tile([HP, N_RECON], F32, tag="dbuf", name="dbuf")[:]
                mbuf_r = mbuf.rearrange("p (d w) -> p d w", w=W)
                nc.vector.tensor_sub(mbuf_r, a_int, recon_r)
                nc.scalar.activation(
                    mbuf, mbuf, AF.Square, accum_out=acc[:, COL_MSE : COL_MSE + 1]
                )

                d_flow_r = d_flow.rearrange("p (c d w) -> p c d w", c=3, w=W)
                for c in range(3):
                    fc = fp.tile([HP, N_FLOW_C], F32, tag="fc", name="fc")[:]
                    nc.sync.dma_start(out=fc, in_=d_flow_r[:, c].rearrange("p d w -> p (d w)"))
                    fc_r = fc.rearrange("p (d w) -> p d w", w=W)

                    # W-axis diffs (innermost)
                    db = fdp.tile([HP, N_RECON], F32, tag="dbuf", name="dbuf")[:]
                    db_x = db.rearrange("p (d w) -> p d w", w=W)[:, :, 0 : W - 1]
                    nc.vector.tensor_sub(
                        db_x, fc_r[:, 0:DQ, 1:W], fc_r[:, 0:DQ, 0 : W - 1]
                    )
                    col = COL_DX + c
                    nc.scalar.activation(
                        db.rearrange("p (d w) -> p d w", w=W)[:, :, 0 : W - 1],
                        db.rearrange("p (d w) -> p d w", w=W)[:, :, 0 : W - 1],
                        AF.Square,
                        accum_out=acc[:, col : col + 1],
                    )

                    # H-axis diffs on the PE: psum = bidiag^T @ fc
                    #   out[m, :] = fc[m+1, :] - fc[m, :]
                    db = fdp.tile([HP, N_RECON], F32, tag="dbuf", name="dbuf")[:]
                    fc_flat = fc_r[:, 0:DQ, :].rearrange("p d w -> p (d w)")
                    for j in range(N_RECON // 512):
                        ps = fpsp.tile([HP, 512], F32, tag="fps", name="fps")[:]
                        nc.tensor.matmul(
                            ps[0 : HP - 1, :],
                            bidiag,
                            fc_flat[:, 512 * j : 512 * (j + 1)],
                            start=True,
                            stop=True,
                        )
                        nc.scalar.copy(
                            db[0 : HP - 1, 512 * j : 512 * (j + 1)], ps[0 : HP - 1, :]
                        )
                    col = COL_DY + c
                    nc.scalar.activation(
                        db[0 : HP - 1, :],
                        db[0 : HP - 1, :],
                        AF.Square,
                        accum_out=acc[0 : HP - 1, col : col + 1],
                    )

                    # D-axis diffs
                    db = fdp.tile([HP, N_RECON], F32, tag="dbuf", name="dbuf")[:]
                    nc.vector.tensor_sub(
                        db,
                        fc_r[:, 1 : DQ + 1, :].rearrange("p d w -> p (d w)"),
                        fc_r[:, 0:DQ, :].rearrange("p d w -> p (d w)"),
                    )
                    col = COL_DZ + c
                    nc.scalar.activation(
                        db, db, AF.Square, accum_out=acc[:, col : col + 1]
                    )

            # ---------------- phase 2: NCC ----------------
            with (
                tc.tile_pool(name="dpadp", bufs=2) as dpp,
                tc.tile_pool(name="boxp", bufs=1) as bxp,
                tc.tile_pool(name="scrp", bufs=1) as scp,
                tc.tile_pool(name="psum", bufs=6, space="PSUM") as psp,
            ):
                inI = sip.tile([HP, N_IN], F32, tag="inI", name="inI")[:]
                nc.sync.dma_start(out=inI, in_=d_warped)
                inI_r = inI.rearrange("p (d w) -> p d w", w=WPAD)

                if USE_F32R:
                    F32R = mybir.dt.float32r
                    bandp_mm = bandp.bitcast(F32R)
                    bandn_mm = bandn.bitcast(F32R)
                else:
                    bandp_mm = bandp
                    bandn_mm = bandn

                for ch in range(NCHUNK):
                    r0 = ch * DC_OUT  # first slab row of this chunk
                    Jc = inJ_r[:, r0 : r0 + DC_IN, :]
                    Ic = inI_r[:, r0 : r0 + DC_IN, :]
                    Jc2 = Jc.rearrange("p d w -> p (d w)")
                    Ic2 = Ic.rearrange("p d w -> p (d w)")

                    boxes = {}
                    for v in ("J", "I", "II", "JJ", "IJ"):
                        # source tensor for this volume
                        if v == "J":
                            src2 = Jc2
                        elif v == "I":
                            src2 = Ic2
                        else:
                            prod = prp.tile([HP, N_CHUNK_IN], F32, tag="prod", name="prod")[:]
                            if v == "II":
                                nc.scalar.activation(prod, Ic2, AF.Square)
                            elif v == "JJ":
                                nc.scalar.activation(prod, Jc2, AF.Square)
                            else:
                                nc.vector.tensor_mul(prod, Ic2, Jc2)
                            src2 = prod

                        # W-axis cumulative sum, split into two chained halves
                        # so the H-box matmuls can start after the first half
                        # (box diff is fused into the matmul via the +/- bands)
                        NH = N_CHUNK_IN // 2          # 1728 = 12 d-rows
                        cum_a = cup.tile([HP, NH], F32, tag="cuma", name="cuma")[:]
                        cum_b = cup.tile([HP, NH], F32, tag="cumb", name="cumb")[:]
                        nc.vector.tensor_tensor_scan(
                            cum_a, src2[:, 0:NH], src2[:, 0:NH],
                            0.0, op0=ALU.add, op1=ALU.bypass,
                        )
                        nc.vector.tensor_tensor_scan(
                            cum_b, src2[:, NH:], src2[:, NH:],
                            cum_a[:, NH - 1 : NH], op0=ALU.add, op1=ALU.bypass,
                        )
                        cum_ar = cum_a.rearrange("p (d w) -> p d w", w=WPAD)
                        cum_br = cum_b.rearrange("p (d w) -> p d w", w=WPAD)

                        # dpad: [p, (w, dp)] with dp innermost, zero pads
                        dpad = dpp.tile([HP, N_DPAD], F32, tag="dpad", name="dpad")[:]
                        dpad_r = dpad.rearrange("p (w dp) -> p w dp", dp=DCPAD)
                        nc.gpsimd.memset(dpad_r[:, :, 0:DPOFF], 0.0)
                        nc.gpsimd.memset(dpad_r[:, :, DPOFF + DC_IN : DCPAD], 0.0)

                        # H-box matmuls with fused W-box difference:
                        #   psum = band^T @ cum[.., w+9] - band^T @ cum[.., w+0]
                        for j in range(N_CHUNK_HB // 512):
                            dlo = 4 * j
                            ps = psp.tile([HP, 512], F32, tag="ps", name="ps")[:]
                            cr = cum_ar if j < 3 else cum_br
                            dl = dlo if j < 3 else dlo - 12
                            rhs9 = cr[:, dl : dl + 4, 9 : 9 + W]
                            rhs0 = cr[:, dl : dl + 4, 0:W]
                            if USE_F32R:
                                rhs9 = rhs9.bitcast(mybir.dt.float32r)
                                rhs0 = rhs0.bitcast(mybir.dt.float32r)
                            nc.tensor.matmul(
                                ps, bandp_mm, rhs9, start=True, stop=False
                            )
                            nc.tensor.matmul(
                                ps, bandn_mm, rhs0, start=False, stop=True
                            )
                            # evacuate into dpad, transposed to (w, dp)
                            ps_wd = ps.rearrange("p (s w) -> p w s", w=W)
                            nc.scalar.copy(
                                dpad_r[:, :, DPOFF + dlo : DPOFF + dlo + 4], ps_wd
                            )

                        # D-axis cumsum + diff -> final 9^3 box sums
                        nc.vector.tensor_tensor_scan(
                            dpad, dpad, dpad, 0.0, op0=ALU.add, op1=ALU.bypass
                        )
                        cumd_r = dpad.rearrange("p (w dp) -> p w dp", dp=DCPAD)
                        B = bxp.tile([HP, N_BOX], F32, tag=f"box{v}", name=f"box{v}")[:]
                        B_r = B.rearrange("p (w d) -> p w d", d=DC_OUT)
                        nc.vector.tensor_sub(
                            B_r,
                            cumd_r[:, :, 9 : 9 + DC_OUT],
                            cumd_r[:, :, 0 : 0 + DC_OUT],
                        )
                        boxes[v] = B

                    # ---- cc math on [128, 2048] box sums ----
                    BJ, BI = boxes["J"], boxes["I"]
                    BII, BJJ, BIJ = boxes["II"], boxes["JJ"], boxes["IJ"]
                    s1 = scp.tile([HP, N_BOX], F32, tag="s1", name="s1")[:]
                    s2 = scp.tile([HP, N_BOX], F32, tag="s2", name="s2")[:]
                    s3 = scp.tile([HP, N_BOX], F32, tag="s3", name="s3")[:]

                    nc.vector.tensor_mul(s1, BI, BJ)
                    nc.vector.scalar_tensor_tensor(
                        s2, s1, -1.0 / WIN3, BIJ, op0=ALU.mult, op1=ALU.add
                    )  # cross
                    nc.scalar.activation(s1, s2, AF.Square)       # cross^2
                    nc.scalar.activation(s2, s1, AF.Ln)           # ln(cross^2)
                    nc.scalar.activation(s1, BI, AF.Square)
                    nc.vector.scalar_tensor_tensor(
                        s3, s1, -1.0 / WIN3, BII, op0=ALU.mult, op1=ALU.add
                    )  # I_var
                    nc.scalar.activation(s1, BJ, AF.Square)
                    nc.vector.scalar_tensor_tensor(
                        BII, s1, -1.0 / WIN3, BJJ, op0=ALU.mult, op1=ALU.add
                    )  # J_var (overwrites dead BII)
                    nc.vector.tensor_mul(s1, s3, BII)             # I_var * J_var
                    nc.scalar.activation(s3, s1, AF.Ln, bias=eps_ap)
                    nc.vector.tensor_sub(s1, s2, s3)
                    col = COL_CC0 + ch
                    nc.scalar.activation(
                        s3, s1, AF.Exp, accum_out=acc[:, col : col + 1]
                    )

            _es.close()
            nc.sync.dma_start(out=d_out, in_=acc)

    nc.compile()
    return nc


def _make_band() -> tuple[np.ndarray, np.ndarray, np.ndarray]:
    k = np.arange(HP)
    band = (np.abs(k[:, None] - k[None, :]) <= 4).astype(np.float32)
    m = np.arange(HP - 1)
    bidiag = np.zeros((HP, HP - 1), np.float32)
    bidiag[m + 1, m] = 1.0
    bidiag[m, m] = -1.0
    return band, -band, bidiag


def _shard_inputs(imgsA, recon_A, warped_BA, flow_BA):
    bandp, bandn, bidiag = _make_band()
    in_maps = []
    for core in range(8):
        b, q = divmod(core, 4)
        d0 = DQ * q

        def slab(vol):
            s = np.zeros((HP, D_IN, WPAD), np.float32)
            lo, hi = d0 - 4, d0 + DQ + 4
            clo, chi = max(lo, 0), min(hi, D_FULL)
            s[:, clo - lo : chi - lo, WOFF : WOFF + W] = np.ascontiguousarray(
                vol[clo:chi].transpose(1, 0, 2)
            )
            return s.reshape(HP, N_IN)

        rec = np.ascontiguousarray(
            recon_A[b, 0, d0 : d0 + DQ].transpose(1, 0, 2)
        ).reshape(HP, N_RECON)

        fl = np.empty((HP, 3, FLOW_D, W), np.float32)
        hi = min(d0 + FLOW_D, D_FULL)
        n = hi - d0
        fl[:, :, :n] = flow_BA[b, :, d0:hi].transpose(2, 0, 1, 3)
        if n < FLOW_D:
            fl[:, :, n:] = fl[:, :, n - 1 : n]

        in_maps.append(
            {
                "imgsA": slab(imgsA[b, 0]),
                "warped": slab(warped_BA[b, 0]),
                "recon": rec,
                "flow": np.ascontiguousarray(fl).reshape(HP, 3 * N_FLOW_C),
                "bandp": bandp,
                "bandn": bandn,
                "bidiag": bidiag,
            }
        )
    return in_maps


def _install_profile_shim():
    """Wire up NTFF profiling under axon when antenv.axon_hooks is absent."""
    try:
        import antenv.axon_hooks  # noqa: F401

        return True
    except ImportError:
        pass
    import contextlib
    import ctypes
    import types

    so_path = "/opt/axon/libaxon_pjrt.so"
    if not os.path.exists(so_path):
        return False
    lib = ctypes.CDLL(so_path)
    if not hasattr(lib, "axon_start_nrt_profile"):
        return False
    lib.axon_start_nrt_profile.argtypes = [
        ctypes.POINTER(ctypes.c_int64),
        ctypes.c_size_t,
    ]
    lib.axon_start_nrt_profile.restype = ctypes.c_int64
    lib.axon_stop_nrt_profile.argtypes = [ctypes.c_char_p]
    lib.axon_stop_nrt_profile.restype = ctypes.c_int64

    @contextlib.contextmanager
    def _hook(output_dir, device_ids):
        import jax

        jax.devices()
        if device_ids:
            ids = (ctypes.c_int64 * len(device_ids))(*device_ids)
            rc = lib.axon_start_nrt_profile(ids, len(device_ids))
        else:
            rc = lib.axon_start_nrt_profile(None, 0)
        if rc != 0:
            raise RuntimeError(f"axon_start_nrt_profile rc={rc}")
        try:
            yield
        finally:
            n = lib.axon_stop_nrt_profile(str(output_dir).encode())
            print(f"ntff profile: {n} file(s) written to {output_dir}")

    mod = types.ModuleType("antenv.axon_hooks")
    mod.get_axon_ntff_profile_hook = lambda: _hook
    mod.set_axon_ntff_profile_hook = lambda h: None
    import antenv

    sys.modules["antenv.axon_hooks"] = mod
    antenv.axon_hooks = mod

    # keep profile artifacts local instead of uploading to fishnet
    import concourse.bass_utils as _bu

    _bu.upload_artifacts = lambda tmpdir: tmpdir
    return True


LAST_EXEC_NS = None
LAST_RESULTS = None


def kernel(imgsA, recon_A, warped_BA, flow_BA):
    global LAST_EXEC_NS, LAST_RESULTS
    if "nc" not in _CACHE:
        _CACHE["nc"] = _build_program()
    nc = _CACHE["nc"]

    in_maps = _shard_inputs(
        np.asarray(imgsA, np.float32),
        np.asarray(recon_A, np.float32),
        np.asarray(warped_BA, np.float32),
        np.asarray(flow_BA, np.float32),
    )
    trace = os.environ.get("GVSL_TRACE", "0") == "1"
    if trace:
        trace = _install_profile_shim()
    tmpdir = os.environ.get("GVSL_TRACE_DIR") or None
    res = run_bass_kernel_spmd(
        nc, in_maps, core_ids=list(range(8)), trace=trace, tmpdir=tmpdir
    )
    LAST_EXEC_NS = res.exec_time_ns
    LAST_RESULTS = res

    cc = mse = dx = dy = dz = 0.0
    for r in res.results:
        o = np.asarray(r["out"], np.float64)
        cc += o[:, COL_CC0].sum() + o[:, COL_CC0 + 1].sum()
        mse += o[:, COL_MSE].sum()
        dx += o[:, COL_DX : COL_DX + 3].sum()
        dy += o[: HP - 1, COL_DY : COL_DY + 3].sum()
        dz += o[:, COL_DZ : COL_DZ + 3].sum()

    n_vox = 2 * 1 * 128 * 128 * 128
    n_d = 2 * 3 * 127 * 128 * 128
    ncc_loss = 1.0 - cc / n_vox
    mse_loss = mse / n_vox
    smooth_loss = (dx / n_d + dy / n_d + dz / n_d) / 3.0
    return (
        np.float32(ncc_loss),
        np.float32(mse_loss),
        np.float32(smooth_loss),
    )


# revision 26
# speedup vs baseline: 2.2037x; 1.0393x over previous
"""GVSL loss (NCC + MSE + smoothness) as a distributed Bass kernel on 8 TRN2 cores.

Sharding: batch(2) x depth-quarters(4) = 8 shards. Each core computes partial
sums for its 32-deep output slab (with 4-voxel halo for the 9^3 box filter);
the final scalar reductions happen on the host.
"""

import os
import sys

for _p in ("/opt/trn_rl_repo",):
    if _p not in sys.path:
        sys.path.insert(0, _p)

import numpy as np

import concourse.bass as bass
import concourse.tile as tile
from concourse import bacc, mybir
from concourse.bass_utils import run_bass_kernel_spmd

F32 = mybir.dt.float32
AF = mybir.ActivationFunctionType
ALU = mybir.AluOpType

HP = 128          # partitions (H axis)
W = 128
D_FULL = 128
DQ = 32           # output depths per core
D_IN = DQ + 8     # slab rows incl. halo
WPAD = 140        # 5 zeros | 128 data | 7 zeros
WOFF = 5
NCHUNK = 2
DC_OUT = DQ // NCHUNK          # 16
DC_IN = DC_OUT + 8             # 24
DCPAD = 26                     # 1 zero | 24 data | 1 zero
DPOFF = 1
FLOW_D = DQ + 1                # 33
WIN3 = 729.0

N_IN = D_IN * WPAD             # 5760
N_CHUNK_IN = DC_IN * WPAD      # 3456
N_CHUNK_HB = DC_IN * W         # 3072  (H-boxed compact, per chunk)
N_DPAD = W * DCPAD             # 4096
N_BOX = W * DC_OUT             # 2048
N_RECON = DQ * W               # 4096
N_FLOW_C = FLOW_D * W          # 4224

# Use the fast fp32 path on the PE for the band matmuls (float32r).
USE_F32R = os.environ.get("GVSL_F32R", "0") == "1"

# acc_all columns
COL_CC0 = 0          # cc sums, 2 chunks x 2 slices -> cols 0..3
COL_MSE = 4
COL_DX = 5           # +c, W-axis diffs (3 channels)
COL_DZ = 8           # +c, D-axis diffs
COL_DY = 12          # +c*8+j, H-axis diffs per psum chunk
ACC_W = 40

_CACHE = {}


def _build_program():
    nc = bacc.Bacc("TRN2", target_bir_lowering=False, debug=False, num_devices=8)

    d_imgsA = nc.dram_tensor("imgsA", [HP, N_IN], F32, kind="ExternalInput").ap()
    d_warped = nc.dram_tensor("warped", [HP, N_IN], F32, kind="ExternalInput").ap()
    d_recon = nc.dram_tensor("recon", [HP, N_RECON], F32, kind="ExternalInput").ap()
    d_flow = nc.dram_tensor("flow", [HP, 3 * N_FLOW_C], F32, kind="ExternalInput").ap()
    d_bandp = nc.dram_tensor("bandp", [HP, HP], F32, kind="ExternalInput").ap()
    d_bandn = nc.dram_tensor("bandn", [HP, HP], F32, kind="ExternalInput").ap()
    d_bidiag = nc.dram_tensor("bidiag", [HP, HP - 1], F32, kind="ExternalInput").ap()
    d_out = nc.dram_tensor("out", [HP, ACC_W], F32, kind="ExternalOutput").ap()

    with tile.TileContext(nc) as tc:
        with tc.tile_pool(name="persist", bufs=1) as pp:
            acc = pp.tile([HP, ACC_W], F32, tag="acc", name="acc")[:]
            eps_ap = pp.tile([HP, 1], F32, tag="epsc", name="epsc")[:]
            nc.gpsimd.memset(eps_ap, 1e-5)
            bandp = pp.tile([HP, HP], F32, tag="bandp", name="bandp")[:]
            bandn = pp.tile([HP, HP], F32, tag="bandn", name="bandn")[:]
            bidiag = pp.tile([HP, HP - 1], F32, tag="bidiag", name="bidiag")[:]
            inJ = pp.tile([HP, N_IN], F32, tag="inJ", name="inJ")[:]
            nc.sync.dma_start(out=bandp, in_=d_bandp)
            nc.sync.dma_start(out=bandn, in_=d_bandn)
            nc.sync.dma_start(out=bidiag, in_=d_bidiag)
            nc.sync.dma_start(out=inJ, in_=d_imgsA)
            inJ_r = inJ.rearrange("p (d w) -> p d w", w=WPAD)

            # NCC front-end pools opened early so their space does not
            # overlap the flow pools: lets NCC scans overlap phase 1.
            from contextlib import ExitStack as _ES
            _es = _ES()
            sip = _es.enter_context(tc.tile_pool(name="srcI", bufs=1))
            prp = _es.enter_context(tc.tile_pool(name="prodp", bufs=2))
            cup = _es.enter_context(tc.tile_pool(name="cump", bufs=2))
            psp = _es.enter_context(tc.tile_pool(name="psum", bufs=1, space="PSUM"))

            # ---------------- phase 1: smoothness + MSE ----------------
            with (
                tc.tile_pool(name="flowp", bufs=2) as fp,
                tc.tile_pool(name="diffp", bufs=2) as fdp,
                tc.tile_pool(name="reconp", bufs=1) as rp,
            ):
                recon = rp.tile([HP, N_RECON], F32, tag="recon", name="recon")[:]
                nc.sync.dma_start(out=recon, in_=d_recon)
                recon_r = recon.rearrange("p (d w) -> p d w", w=W)

                # MSE = sum((imgsA - recon)^2) over the interior slab
                a_int = inJ_r[:, 4 : 4 + DQ, WOFF : WOFF + W]
                mbuf = fdp.tile([HP, N_RECON], F32, tag="dbuf", name="dbuf")[:]
                mbuf_r = mbuf.rearrange("p (d w) -> p d w", w=W)
                nc.vector.tensor_sub(mbuf_r, a_int, recon_r)
                nc.scalar.activation(
                    mbuf, mbuf, AF.Square, accum_out=acc[:, COL_MSE : COL_MSE + 1]
                )

                d_flow_r = d_flow.rearrange("p (c d w) -> p c d w", c=3, w=W)
                for c in range(3):
                    fc = fp.tile([HP, N_FLOW_C], F32, tag="fc", name="fc")[:]
                    nc.sync.dma_start(out=fc, in_=d_flow_r[:, c].rearrange("p d w -> p (d w)"))
                    fc_r = fc.rearrange("p (d w) -> p d w", w=W)

                    # W-axis diffs (innermost)
                    db = fdp.tile([HP, N_RECON], F32, tag="dbuf", name="dbuf")[:]
                    db_x = db.rearrange("p (d w) -> p d w", w=W)[:, :, 0 : W - 1]
                    nc.vector.tensor_sub(
                        db_x, fc_r[:, 0:DQ, 1:W], fc_r[:, 0:DQ, 0 : W - 1]
                    )
                    col = COL_DX + c
                    nc.scalar.activation(
                        db.rearrange("p (d w) -> p d w", w=W)[:, :, 0 : W - 1],
                        db.rearrange("p (d w) -> p d w", w=W)[:, :, 0 : W - 1],
                        AF.Square,
                        accum_out=acc[:, col : col + 1],
                    )

                    # H-axis diffs on the PE: psum = bidiag^T @ fc
                    #   out[m, :] = fc[m+1, :] - fc[m, :]; square+accumulate
                    #   in place on PSUM (one acc column per psum chunk)
                    fc_flat = fc_r[:, 0:DQ, :].rearrange("p d w -> p (d w)")
                    for j in range(N_RECON // 512):
                        ps = psp.tile([HP, 512], F32, tag="fps", name="fps", bufs=2)[:]
                        nc.tensor.matmul(
                            ps[0 : HP - 1, :],
                            bidiag,
                            fc_flat[:, 512 * j : 512 * (j + 1)],
                            start=True,
                            stop=True,
                        )
                        col = COL_DY + c * 8 + j
                        nc.scalar.activation(
                            ps[0 : HP - 1, :],
                            ps[0 : HP - 1, :],
                            AF.Square,
                            accum_out=acc[0 : HP - 1, col : col + 1],
                        )

                    # D-axis diffs
                    db = fdp.tile([HP, N_RECON], F32, tag="dbuf", name="dbuf")[:]
                    nc.vector.tensor_sub(
                        db,
                        fc_r[:, 1 : DQ + 1, :].rearrange("p d w -> p (d w)"),
                        fc_r[:, 0:DQ, :].rearrange("p d w -> p (d w)"),
                    )
                    col = COL_DZ + c
                    nc.scalar.activation(
                        db, db, AF.Square, accum_out=acc[:, col : col + 1]
                    )

            # ---------------- phase 2: NCC ----------------
            with (
                tc.tile_pool(name="dpadp", bufs=2) as dpp,
                tc.tile_pool(name="boxp", bufs=1) as bxp,
                tc.tile_pool(name="scrp", bufs=1) as scp,
            ):
                inI = sip.tile([HP, N_IN], F32, tag="inI", name="inI")[:]
                nc.sync.dma_start(out=inI, in_=d_warped)
                inI_r = inI.rearrange("p (d w) -> p d w", w=WPAD)

                if USE_F32R:
                    F32R = mybir.dt.float32r
                    bandp_mm = bandp.bitcast(F32R)
                    bandn_mm = bandn.bitcast(F32R)
                else:
                    bandp_mm = bandp
                    bandn_mm = bandn

                for ch in range(NCHUNK):
                    r0 = ch * DC_OUT  # first slab row of this chunk
                    Jc = inJ_r[:, r0 : r0 + DC_IN, :]
                    Ic = inI_r[:, r0 : r0 + DC_IN, :]
                    Jc2 = Jc.rearrange("p d w -> p (d w)")
                    Ic2 = Ic.rearrange("p d w -> p (d w)")

                    boxes = {}
                    for v in ("J", "I", "II", "JJ", "IJ"):
                        # source tensor for this volume
                        if v == "J":
                            src2 = Jc2
                        elif v == "I":
                            src2 = Ic2
                        else:
                            prod = prp.tile([HP, N_CHUNK_IN], F32, tag="prod", name="prod")[:]
                            if v == "II":
                                nc.scalar.activation(prod, Ic2, AF.Square)
                            elif v == "JJ":
                                nc.scalar.activation(prod, Jc2, AF.Square)
                            else:
                                nc.vector.tensor_mul(prod, Ic2, Jc2)
                            src2 = prod

                        # W-axis cumulative sum, split into two chained halves
                        # so the H-box matmuls can start after the first half
                        # (box diff is fused into the matmul via the +/- bands)
                        NH = N_CHUNK_IN // 2          # 1728 = 12 d-rows
                        cum_a = cup.tile([HP, NH], F32, tag="cuma", name="cuma")[:]
                        cum_b = cup.tile([HP, NH], F32, tag="cumb", name="cumb")[:]
                        nc.vector.tensor_tensor_scan(
                            cum_a, src2[:, 0:NH], src2[:, 0:NH],
                            0.0, op0=ALU.add, op1=ALU.bypass,
                        )
                        nc.vector.tensor_tensor_scan(
                            cum_b, src2[:, NH:], src2[:, NH:],
                            cum_a[:, NH - 1 : NH], op0=ALU.add, op1=ALU.bypass,
                        )
                        cum_ar = cum_a.rearrange("p (d w) -> p d w", w=WPAD)
                        cum_br = cum_b.rearrange("p (d w) -> p d w", w=WPAD)

                        # dpad: [p, (w, dp)] with dp innermost, zero pads
                        dpad = dpp.tile([HP, N_DPAD], F32, tag="dpad", name="dpad")[:]
                        dpad_r = dpad.rearrange("p (w dp) -> p w dp", dp=DCPAD)
                        nc.gpsimd.memset(dpad_r[:, :, 0:DPOFF], 0.0)
                        nc.gpsimd.memset(dpad_r[:, :, DPOFF + DC_IN : DCPAD], 0.0)

                        # H-box matmuls with fused W-box difference:
                        #   psum = band^T @ cum[.., w+9] - band^T @ cum[.., w+0]
                        for j in range(N_CHUNK_HB // 512):
                            dlo = 4 * j
                            ps = psp.tile([HP, 512], F32, tag="ps", name="ps", bufs=6)[:]
                            cr = cum_ar if j < 3 else cum_br
                            dl = dlo if j < 3 else dlo - 12
                            rhs9 = cr[:, dl : dl + 4, 9 : 9 + W]
                            rhs0 = cr[:, dl : dl + 4, 0:W]
                            if USE_F32R:
                                rhs9 = rhs9.bitcast(mybir.dt.float32r)
                                rhs0 = rhs0.bitcast(mybir.dt.float32r)
                            nc.tensor.matmul(
                                ps, bandp_mm, rhs9, start=True, stop=False
                            )
                            nc.tensor.matmul(
                                ps, bandn_mm, rhs0, start=False, stop=True
                            )
                            # evacuate into dpad, transposed to (w, dp)
                            ps_wd = ps.rearrange("p (s w) -> p w s", w=W)
                            nc.scalar.copy(
                                dpad_r[:, :, DPOFF + dlo : DPOFF + dlo + 4], ps_wd
                            )

                        # D-axis cumsum + diff -> final 9^3 box sums
                        nc.vector.tensor_tensor_scan(
                            dpad, dpad, dpad, 0.0, op0=ALU.add, op1=ALU.bypass
                        )
                        cumd_r = dpad.rearrange("p (w dp) -> p w dp", dp=DCPAD)
                        B = bxp.tile([HP, N_BOX], F32, tag=f"box{v}", name=f"box{v}")[:]
                        B_r = B.rearrange("p (w d) -> p w d", d=DC_OUT)
                        nc.vector.tensor_sub(
                            B_r,
                            cumd_r[:, :, 9 : 9 + DC_OUT],
                            cumd_r[:, :, 0 : 0 + DC_OUT],
                        )
                        boxes[v] = B

                    # ---- cc math on [128, 2048] box sums, in 2 slices
                    # so DVE and ACT pipeline across slices ----
                    NS = N_BOX // 2
                    for sl in range(2):
                        lo, hi = sl * NS, (sl + 1) * NS
                        BJ = boxes["J"][:, lo:hi]
                        BI = boxes["I"][:, lo:hi]
                        BII = boxes["II"][:, lo:hi]
                        BJJ = boxes["JJ"][:, lo:hi]
                        BIJ = boxes["IJ"][:, lo:hi]
                        s1 = scp.tile([HP, NS], F32, tag="s1", name="s1", bufs=2)[:]
                        s2 = scp.tile([HP, NS], F32, tag="s2", name="s2", bufs=2)[:]
                        s3 = scp.tile([HP, NS], F32, tag="s3", name="s3", bufs=2)[:]

                        nc.vector.tensor_mul(s1, BI, BJ)
                        nc.vector.scalar_tensor_tensor(
                            s2, s1, -1.0 / WIN3, BIJ, op0=ALU.mult, op1=ALU.add
                        )  # cross
                        nc.scalar.activation(s1, s2, AF.Square)   # cross^2
                        nc.scalar.activation(s2, s1, AF.Ln)       # ln(cross^2)
                        nc.scalar.activation(s1, BI, AF.Square)
                        nc.vector.scalar_tensor_tensor(
                            s3, s1, -1.0 / WIN3, BII, op0=ALU.mult, op1=ALU.add
                        )  # I_var
                        nc.scalar.activation(s1, BJ, AF.Square)
                        nc.vector.scalar_tensor_tensor(
                            BII, s1, -1.0 / WIN3, BJJ, op0=ALU.mult, op1=ALU.add
                        )  # J_var (overwrites dead BII)
                        nc.vector.tensor_mul(s1, s3, BII)         # I_var * J_var
                        nc.scalar.activation(s3, s1, AF.Ln, bias=eps_ap)
                        nc.vector.tensor_sub(s1, s2, s3)
                        col = COL_CC0 + ch * 2 + sl
                        nc.scalar.activation(
                            s3, s1, AF.Exp, accum_out=acc[:, col : col + 1]
                        )

            _es.close()
            nc.sync.dma_start(out=d_out, in_=acc)

    nc.compile()
    return nc


def _make_band() -> tuple[np.ndarray, np.ndarray, np.ndarray]:
    k = np.arange(HP)
    band = (np.abs(k[:, None] - k[None, :]) <= 4).astype(np.float32)
    m = np.arange(HP - 1)
    bidiag = np.zeros((HP, HP - 1), np.float32)
    bidiag[m + 1, m] = 1.0
    bidiag[m, m] = -1.0
    return band, -band, bidiag


def _shard_inputs(imgsA, recon_A, warped_BA, flow_BA):
    bandp, bandn, bidiag = _make_band()
    in_maps = []
    for core in range(8):
        b, q = divmod(core, 4)
        d0 = DQ * q

        def slab(vol):
            s = np.zeros((HP, D_IN, WPAD), np.float32)
            lo, hi = d0 - 4, d0 + DQ + 4
            clo, chi = max(lo, 0), min(hi, D_FULL)
            s[:, clo - lo : chi - lo, WOFF : WOFF + W] = np.ascontiguousarray(
                vol[clo:chi].transpose(1, 0, 2)
            )
            return s.reshape(HP, N_IN)

        rec = np.ascontiguousarray(
            recon_A[b, 0, d0 : d0 + DQ].transpose(1, 0, 2)
        ).reshape(HP, N_RECON)

        fl = np.empty((HP, 3, FLOW_D, W), np.float32)
        hi = min(d0 + FLOW_D, D_FULL)
        n = hi - d0
        fl[:, :, :n] = flow_BA[b, :, d0:hi].transpose(2, 0, 1, 3)
        if n < FLOW_D:
            fl[:, :, n:] = fl[:, :, n - 1 : n]

        in_maps.append(
            {
                "imgsA": slab(imgsA[b, 0]),
                "warped": slab(warped_BA[b, 0]),
                "recon": rec,
                "flow": np.ascontiguousarray(fl).reshape(HP, 3 * N_FLOW_C),
                "bandp": bandp,
                "bandn": bandn,
                "bidiag": bidiag,
            }
        )
    return in_maps


def _install_profile_shim():
    """Wire up NTFF profiling under axon when antenv.axon_hooks is absent."""
    try:
        import antenv.axon_hooks  # noqa: F401

        return True
    except ImportError:
        pass
    import contextlib
    import ctypes
    import types

    so_path = "/opt/axon/libaxon_pjrt.so"
    if not os.path.exists(so_path):
        return False
    lib = ctypes.CDLL(so_path)
    if not hasattr(lib, "axon_start_nrt_profile"):
        return False
    lib.axon_start_nrt_profile.argtypes = [
        ctypes.POINTER(ctypes.c_int64),
        ctypes.c_size_t,
    ]
    lib.axon_start_nrt_profile.restype = ctypes.c_int64
    lib.axon_stop_nrt_profile.argtypes = [ctypes.c_char_p]
    lib.axon_stop_nrt_profile.restype = ctypes.c_int64

    @contextlib.contextmanager
    def _hook(output_dir, device_ids):
        import jax

        jax.devices()
        if device_ids:
            ids = (ctypes.c_int64 * len(device_ids))(*device_ids)
            rc = lib.axon_start_nrt_profile(ids, len(device_ids))
        else:
            rc = lib.axon_start_nrt_profile(None, 0)
        if rc != 0:
            raise RuntimeError(f"axon_start_nrt_profile rc={rc}")
        try:
            yield
        finally:
            n = lib.axon_stop_nrt_profile(str(output_dir).encode())
            print(f"ntff profile: {n} file(s) written to {output_dir}")

    mod = types.ModuleType("antenv.axon_hooks")
    mod.get_axon_ntff_profile_hook = lambda: _hook
    mod.set_axon_ntff_profile_hook = lambda h: None
    import antenv

    sys.modules["antenv.axon_hooks"] = mod
    antenv.axon_hooks = mod

    # keep profile artifacts local instead of uploading to fishnet
    import concourse.bass_utils as _bu

    _bu.upload_artifacts = lambda tmpdir: tmpdir
    return True


LAST_EXEC_NS = None
LAST_RESULTS = None


def kernel(imgsA, recon_A, warped_BA, flow_BA):
    global LAST_EXEC_NS, LAST_RESULTS
    if "nc" not in _CACHE:
        _CACHE["nc"] = _build_program()
    nc = _CACHE["nc"]

    in_maps = _shard_inputs(
        np.asarray(imgsA, np.float32),
        np.asarray(recon_A, np.float32),
        np.asarray(warped_BA, np.float32),
        np.asarray(flow_BA, np.float32),
    )
    trace = os.environ.get("GVSL_TRACE", "0") == "1"
    if trace:
        trace = _install_profile_shim()
    tmpdir = os.environ.get("GVSL_TRACE_DIR") or None
    res = run_bass_kernel_spmd(
        nc, in_maps, core_ids=list(range(8)), trace=trace, tmpdir=tmpdir
    )
    LAST_EXEC_NS = res.exec_time_ns
    LAST_RESULTS = res

    cc = mse = dx = dy = dz = 0.0
    for r in res.results:
        o = np.asarray(r["out"], np.float64)
        cc += o[:, COL_CC0 : COL_CC0 + 4].sum()
        mse += o[:, COL_MSE].sum()
        dx += o[:, COL_DX : COL_DX + 3].sum()
        dy += o[: HP - 1, COL_DY : COL_DY + 24].sum()
        dz += o[:, COL_DZ : COL_DZ + 3].sum()

    n_vox = 2 * 1 * 128 * 128 * 128
    n_d = 2 * 3 * 127 * 128 * 128
    ncc_loss = 1.0 - cc / n_vox
    mse_loss = mse / n_vox
    smooth_loss = (dx / n_d + dy / n_d + dz / n_d) / 3.0
    return (
        np.float32(ncc_loss),
        np.float32(mse_loss),
        np.float32(smooth_loss),
    )


# revision 27
# speedup vs baseline: 2.2652x; 1.0279x over previous
"""GVSL loss (NCC + MSE + smoothness) as a distributed Bass kernel on 8 TRN2 cores.

Sharding: batch(2) x depth-quarters(4) = 8 shards. Each core computes partial
sums for its 32-deep output slab (with 4-voxel halo for the 9^3 box filter);
the final scalar reductions happen on the host.
"""

import os
import sys

for _p in ("/opt/trn_rl_repo",):
    if _p not in sys.path:
        sys.path.insert(0, _p)

import numpy as np

import concourse.bass as bass
import concourse.tile as tile
from concourse import bacc, mybir
from concourse.bass_utils import run_bass_kernel_spmd

F32 = mybir.dt.float32
AF = mybir.ActivationFunctionType
ALU = mybir.AluOpType

HP = 128          # partitions (H axis)
W = 128
D_FULL = 128
DQ = 32           # output depths per core
D_IN = DQ + 8     # slab rows incl. halo
WPAD = 140        # 5 zeros | 128 data | 7 zeros
WOFF = 5
NCHUNK = 2
DC_OUT = DQ // NCHUNK          # 16
DC_IN = DC_OUT + 8             # 24
DCPAD = 26                     # 1 zero | 24 data | 1 zero
DPOFF = 1
FLOW_D = DQ + 1                # 33
WIN3 = 729.0

N_IN = D_IN * WPAD             # 5760
N_CHUNK_IN = DC_IN * WPAD      # 3456
N_CHUNK_HB = DC_IN * W         # 3072  (H-boxed compact, per chunk)
N_DPAD = W * DCPAD             # 4096
N_BOX = W * DC_OUT             # 2048
N_RECON = DQ * W               # 4096
N_FLOW_C = FLOW_D * W          # 4224

# Use the fast fp32 path on the PE for the band matmuls (float32r).
USE_F32R = os.environ.get("GVSL_F32R", "0") == "1"

# acc_all columns
COL_CC0 = 0          # cc sums, 2 chunks x 2 slices -> cols 0..3
COL_MSE = 4
COL_DX = 5           # +c, W-axis diffs (3 channels)
COL_DZ = 8           # +c, D-axis diffs
COL_DY = 12          # +c*8+j, H-axis diffs per psum chunk
ACC_W = 40

_CACHE = {}


def _build_program():
    nc = bacc.Bacc("TRN2", target_bir_lowering=False, debug=False, num_devices=8)

    d_imgsA = nc.dram_tensor("imgsA", [HP, N_IN], F32, kind="ExternalInput").ap()
    d_warped = nc.dram_tensor("warped", [HP, N_IN], F32, kind="ExternalInput").ap()
    d_recon = nc.dram_tensor("recon", [HP, N_RECON], F32, kind="ExternalInput").ap()
    d_flow = nc.dram_tensor("flow", [HP, 3 * N_FLOW_C], F32, kind="ExternalInput").ap()
    d_bandp = nc.dram_tensor("bandp", [HP, HP], F32, kind="ExternalInput").ap()
    d_bandn = nc.dram_tensor("bandn", [HP, HP], F32, kind="ExternalInput").ap()
    d_bidiag = nc.dram_tensor("bidiag", [HP, HP - 1], F32, kind="ExternalInput").ap()
    d_out = nc.dram_tensor("out", [HP, ACC_W], F32, kind="ExternalOutput").ap()

    with tile.TileContext(nc) as tc:
        with tc.tile_pool(name="persist", bufs=1) as pp:
            acc = pp.tile([HP, ACC_W], F32, tag="acc", name="acc")[:]
            eps_ap = pp.tile([HP, 1], F32, tag="epsc", name="epsc")[:]
            nc.gpsimd.memset(eps_ap, 1e-5)
            bandp = pp.tile([HP, HP], F32, tag="bandp", name="bandp")[:]
            bandn = pp.tile([HP, HP], F32, tag="bandn", name="bandn")[:]
            bidiag = pp.tile([HP, HP - 1], F32, tag="bidiag", name="bidiag")[:]
            inJ = pp.tile([HP, N_IN], F32, tag="inJ", name="inJ")[:]
            nc.sync.dma_start(out=bandp, in_=d_bandp)
            nc.sync.dma_start(out=bandn, in_=d_bandn)
            nc.sync.dma_start(out=bidiag, in_=d_bidiag)
            NJ0 = DC_IN * WPAD  # first-chunk rows first so scans start early
            nc.sync.dma_start(out=inJ[:, 0:NJ0], in_=d_imgsA[:, 0:NJ0])
            nc.sync.dma_start(out=inJ[:, NJ0:], in_=d_imgsA[:, NJ0:])
            inJ_r = inJ.rearrange("p (d w) -> p d w", w=WPAD)

            # NCC front-end pools opened early so their space does not
            # overlap the flow pools: lets NCC scans overlap phase 1.
            from contextlib import ExitStack as _ES
            _es = _ES()
            sip = _es.enter_context(tc.tile_pool(name="srcI", bufs=1))
            prp = _es.enter_context(tc.tile_pool(name="prodp", bufs=2))
            cup = _es.enter_context(tc.tile_pool(name="cump", bufs=2))
            psp = _es.enter_context(tc.tile_pool(name="psum", bufs=1, space="PSUM"))

            # ---------------- phase 1: smoothness + MSE ----------------
            with (
                tc.tile_pool(name="flowp", bufs=2) as fp,
                tc.tile_pool(name="diffp", bufs=2) as fdp,
                tc.tile_pool(name="reconp", bufs=1) as rp,
            ):
                recon = rp.tile([HP, N_RECON], F32, tag="recon", name="recon")[:]
                nc.sync.dma_start(out=recon, in_=d_recon)
                recon_r = recon.rearrange("p (d w) -> p d w", w=W)

                # MSE = sum((imgsA - recon)^2) over the interior slab
                a_int = inJ_r[:, 4 : 4 + DQ, WOFF : WOFF + W]
                mbuf = fdp.tile([HP, N_RECON], F32, tag="dbuf", name="dbuf")[:]
                mbuf_r = mbuf.rearrange("p (d w) -> p d w", w=W)
                nc.vector.tensor_sub(mbuf_r, a_int, recon_r)
                nc.scalar.activation(
                    mbuf, mbuf, AF.Square, accum_out=acc[:, COL_MSE : COL_MSE + 1]
                )

                d_flow_r = d_flow.rearrange("p (c d w) -> p c d w", c=3, w=W)
                for c in range(3):
                    fc = fp.tile([HP, N_FLOW_C], F32, tag="fc", name="fc")[:]
                    nc.sync.dma_start(out=fc, in_=d_flow_r[:, c].rearrange("p d w -> p (d w)"))
                    fc_r = fc.rearrange("p (d w) -> p d w", w=W)

                    # W-axis diffs (innermost)
                    db = fdp.tile([HP, N_RECON], F32, tag="dbuf", name="dbuf")[:]
                    db_x = db.rearrange("p (d w) -> p d w", w=W)[:, :, 0 : W - 1]
                    nc.vector.tensor_sub(
                        db_x, fc_r[:, 0:DQ, 1:W], fc_r[:, 0:DQ, 0 : W - 1]
                    )
                    col = COL_DX + c
                    nc.scalar.activation(
                        db.rearrange("p (d w) -> p d w", w=W)[:, :, 0 : W - 1],
                        db.rearrange("p (d w) -> p d w", w=W)[:, :, 0 : W - 1],
                        AF.Square,
                        accum_out=acc[:, col : col + 1],
                    )

                    # H-axis diffs on the PE: psum = bidiag^T @ fc
                    #   out[m, :] = fc[m+1, :] - fc[m, :]; square+accumulate
                    #   in place on PSUM (one acc column per psum chunk)
                    fc_flat = fc_r[:, 0:DQ, :].rearrange("p d w -> p (d w)")
                    for j in range(N_RECON // 512):
                        ps = psp.tile([HP, 512], F32, tag="fps", name="fps", bufs=2)[:]
                        nc.tensor.matmul(
                            ps[0 : HP - 1, :],
                            bidiag,
                            fc_flat[:, 512 * j : 512 * (j + 1)],
                            start=True,
                            stop=True,
                        )
                        col = COL_DY + c * 8 + j
                        nc.scalar.activation(
                            ps[0 : HP - 1, :],
                            ps[0 : HP - 1, :],
                            AF.Square,
                            accum_out=acc[0 : HP - 1, col : col + 1],
                        )

                    # D-axis diffs
                    db = fdp.tile([HP, N_RECON], F32, tag="dbuf", name="dbuf")[:]
                    nc.vector.tensor_sub(
                        db,
                        fc_r[:, 1 : DQ + 1, :].rearrange("p d w -> p (d w)"),
                        fc_r[:, 0:DQ, :].rearrange("p d w -> p (d w)"),
                    )
                    col = COL_DZ + c
                    nc.scalar.activation(
                        db, db, AF.Square, accum_out=acc[:, col : col + 1]
                    )

            # ---------------- phase 2: NCC ----------------
            with (
                tc.tile_pool(name="dpadp", bufs=2) as dpp,
                tc.tile_pool(name="boxp", bufs=1) as bxp,
                tc.tile_pool(name="scrp", bufs=1) as scp,
            ):
                inI = sip.tile([HP, N_IN], F32, tag="inI", name="inI")[:]
                nc.sync.dma_start(out=inI[:, 0 : DC_IN * WPAD], in_=d_warped[:, 0 : DC_IN * WPAD])
                nc.sync.dma_start(out=inI[:, DC_IN * WPAD :], in_=d_warped[:, DC_IN * WPAD :])
                inI_r = inI.rearrange("p (d w) -> p d w", w=WPAD)

                if USE_F32R:
                    F32R = mybir.dt.float32r
                    bandp_mm = bandp.bitcast(F32R)
                    bandn_mm = bandn.bitcast(F32R)
                else:
                    bandp_mm = bandp
                    bandn_mm = bandn

                for ch in range(NCHUNK):
                    r0 = ch * DC_OUT  # first slab row of this chunk
                    Jc = inJ_r[:, r0 : r0 + DC_IN, :]
                    Ic = inI_r[:, r0 : r0 + DC_IN, :]
                    Jc2 = Jc.rearrange("p d w -> p (d w)")
                    Ic2 = Ic.rearrange("p d w -> p (d w)")

                    boxes = {}
                    for v in ("J", "I", "II", "JJ", "IJ"):
                        # source tensor for this volume
                        if v == "J":
                            src2 = Jc2
                        elif v == "I":
                            src2 = Ic2
                        else:
                            prod = prp.tile([HP, N_CHUNK_IN], F32, tag="prod", name="prod")[:]
                            if v == "II":
                                nc.scalar.activation(prod, Ic2, AF.Square)
                            elif v == "JJ":
                                nc.scalar.activation(prod, Jc2, AF.Square)
                            else:
                                nc.vector.tensor_mul(prod, Ic2, Jc2)
                            src2 = prod

                        # W-axis cumulative sum, split into two chained halves
                        # so the H-box matmuls can start after the first half
                        # (box diff is fused into the matmul via the +/- bands)
                        NH = N_CHUNK_IN // 2          # 1728 = 12 d-rows
                        cum_a = cup.tile([HP, NH], F32, tag="cuma", name="cuma")[:]
                        cum_b = cup.tile([HP, NH], F32, tag="cumb", name="cumb")[:]
                        nc.vector.tensor_tensor_scan(
                            cum_a, src2[:, 0:NH], src2[:, 0:NH],
                            0.0, op0=ALU.add, op1=ALU.bypass,
                        )
                        nc.vector.tensor_tensor_scan(
                            cum_b, src2[:, NH:], src2[:, NH:],
                            cum_a[:, NH - 1 : NH], op0=ALU.add, op1=ALU.bypass,
                        )
                        cum_ar = cum_a.rearrange("p (d w) -> p d w", w=WPAD)
                        cum_br = cum_b.rearrange("p (d w) -> p d w", w=WPAD)

                        # dpad: [p, (w, dp)] with dp innermost, zero pads
                        dpad = dpp.tile([HP, N_DPAD], F32, tag="dpad", name="dpad")[:]
                        dpad_r = dpad.rearrange("p (w dp) -> p w dp", dp=DCPAD)
                        nc.gpsimd.memset(dpad_r[:, :, 0:DPOFF], 0.0)
                        nc.gpsimd.memset(dpad_r[:, :, DPOFF + DC_IN : DCPAD], 0.0)

                        # H-box matmuls with fused W-box difference:
                        #   psum = band^T @ cum[.., w+9] - band^T @ cum[.., w+0]
                        for j in range(N_CHUNK_HB // 512):
                            dlo = 4 * j
                            ps = psp.tile([HP, 512], F32, tag="ps", name="ps", bufs=6)[:]
                            cr = cum_ar if j < 3 else cum_br
                            dl = dlo if j < 3 else dlo - 12
                            rhs9 = cr[:, dl : dl + 4, 9 : 9 + W]
                            rhs0 = cr[:, dl : dl + 4, 0:W]
                            if USE_F32R:
                                rhs9 = rhs9.bitcast(mybir.dt.float32r)
                                rhs0 = rhs0.bitcast(mybir.dt.float32r)
                            nc.tensor.matmul(
                                ps, bandp_mm, rhs9, start=True, stop=False
                            )
                            nc.tensor.matmul(
                                ps, bandn_mm, rhs0, start=False, stop=True
                            )
                            # evacuate into dpad, transposed to (w, dp)
                            ps_wd = ps.rearrange("p (s w) -> p w s", w=W)
                            nc.scalar.copy(
                                dpad_r[:, :, DPOFF + dlo : DPOFF + dlo + 4], ps_wd
                            )

                        # D-axis cumsum + diff -> final 9^3 box sums
                        nc.vector.tensor_tensor_scan(
                            dpad, dpad, dpad, 0.0, op0=ALU.add, op1=ALU.bypass
                        )
                        cumd_r = dpad.rearrange("p (w dp) -> p w dp", dp=DCPAD)
                        B = bxp.tile([HP, N_BOX], F32, tag=f"box{v}", name=f"box{v}")[:]
                        B_r = B.rearrange("p (w d) -> p w d", d=DC_OUT)
                        nc.vector.tensor_sub(
                            B_r,
                            cumd_r[:, :, 9 : 9 + DC_OUT],
                            cumd_r[:, :, 0 : 0 + DC_OUT],
                        )
                        boxes[v] = B

                    # ---- cc math on [128, 2048] box sums, in 2 slices
                    # so DVE and ACT pipeline across slices ----
                    NS = N_BOX // 2
                    for sl in range(2):
                        lo, hi = sl * NS, (sl + 1) * NS
                        BJ = boxes["J"][:, lo:hi]
                        BI = boxes["I"][:, lo:hi]
                        BII = boxes["II"][:, lo:hi]
                        BJJ = boxes["JJ"][:, lo:hi]
                        BIJ = boxes["IJ"][:, lo:hi]
                        s1 = scp.tile([HP, NS], F32, tag="s1", name="s1", bufs=2)[:]
                        s2 = scp.tile([HP, NS], F32, tag="s2", name="s2", bufs=2)[:]
                        s3 = scp.tile([HP, NS], F32, tag="s3", name="s3", bufs=2)[:]

                        nc.vector.tensor_mul(s1, BI, BJ)
                        nc.vector.scalar_tensor_tensor(
                            s2, s1, -1.0 / WIN3, BIJ, op0=ALU.mult, op1=ALU.add
                        )  # cross
                        nc.scalar.activation(s1, s2, AF.Square)   # cross^2
                        nc.scalar.activation(s2, s1, AF.Ln)       # ln(cross^2)
                        nc.scalar.activation(s1, BI, AF.Square)
                        nc.vector.scalar_tensor_tensor(
                            s3, s1, -1.0 / WIN3, BII, op0=ALU.mult, op1=ALU.add
                        )  # I_var
                        nc.scalar.activation(s1, BJ, AF.Square)
                        nc.vector.scalar_tensor_tensor(
                            BII, s1, -1.0 / WIN3, BJJ, op0=ALU.mult, op1=ALU.add
                        )  # J_var (overwrites dead BII)
                        nc.vector.tensor_mul(s1, s3, BII)         # I_var * J_var
                        nc.scalar.activation(s3, s1, AF.Ln, bias=eps_ap)
                        nc.vector.tensor_sub(s1, s2, s3)
                        col = COL_CC0 + ch * 2 + sl
                        nc.scalar.activation(
                            s3, s1, AF.Exp, accum_out=acc[:, col : col + 1]
                        )

            _es.close()
            nc.sync.dma_start(out=d_out, in_=acc)

    nc.compile()
    return nc


def _make_band() -> tuple[np.ndarray, np.ndarray, np.ndarray]:
    k = np.arange(HP)
    band = (np.abs(k[:, None] - k[None, :]) <= 4).astype(np.float32)
    m = np.arange(HP - 1)
    bidiag = np.zeros((HP, HP - 1), np.float32)
    bidiag[m + 1, m] = 1.0
    bidiag[m, m] = -1.0
    return band, -band, bidiag


def _shard_inputs(imgsA, recon_A, warped_BA, flow_BA):
    bandp, bandn, bidiag = _make_band()
    in_maps = []
    for core in range(8):
        b, q = divmod(core, 4)
        d0 = DQ * q

        def slab(vol):
            s = np.zeros((HP, D_IN, WPAD), np.float32)
            lo, hi = d0 - 4, d0 + DQ + 4
            clo, chi = max(lo, 0), min(hi, D_FULL)
            s[:, clo - lo : chi - lo, WOFF : WOFF + W] = np.ascontiguousarray(
                vol[clo:chi].transpose(1, 0, 2)
            )
            return s.reshape(HP, N_IN)

        rec = np.ascontiguousarray(
            recon_A[b, 0, d0 : d0 + DQ].transpose(1, 0, 2)
        ).reshape(HP, N_RECON)

        fl = np.empty((HP, 3, FLOW_D, W), np.float32)
        hi = min(d0 + FLOW_D, D_FULL)
        n = hi - d0
        fl[:, :, :n] = flow_BA[b, :, d0:hi].transpose(2, 0, 1, 3)
        if n < FLOW_D:
            fl[:, :, n:] = fl[:, :, n - 1 : n]

        in_maps.append(
            {
                "imgsA": slab(imgsA[b, 0]),
                "warped": slab(warped_BA[b, 0]),
                "recon": rec,
                "flow": np.ascontiguousarray(fl).reshape(HP, 3 * N_FLOW_C),
                "bandp": bandp,
                "bandn": bandn,
                "bidiag": bidiag,
            }
        )
    return in_maps


def _install_profile_shim():
    """Wire up NTFF profiling under axon when antenv.axon_hooks is absent."""
    try:
        import antenv.axon_hooks  # noqa: F401

        return True
    except ImportError:
        pass
    import contextlib
    import ctypes
    import types

    so_path = "/opt/axon/libaxon_pjrt.so"
    if not os.path.exists(so_path):
        return False
    lib = ctypes.CDLL(so_path)
    if not hasattr(lib, "axon_start_nrt_profile"):
        return False
    lib.axon_start_nrt_profile.argtypes = [
        ctypes.POINTER(ctypes.c_int64),
        ctypes.c_size_t,
    ]
    lib.axon_start_nrt_profile.restype = ctypes.c_int64
    lib.axon_stop_nrt_profile.argtypes = [ctypes.c_char_p]
    lib.axon_stop_nrt_profile.restype = ctypes.c_int64

    @contextlib.contextmanager
    def _hook(output_dir, device_ids):
        import jax

        jax.devices()
        if device_ids:
            ids = (ctypes.c_int64 * len(device_ids))(*device_ids)
            rc = lib.axon_start_nrt_profile(ids, len(device_ids))
        else:
            rc = lib.axon_start_nrt_profile(None, 0)
        if rc != 0:
            raise RuntimeError(f"axon_start_nrt_profile rc={rc}")
        try:
            yield
        finally:
            n = lib.axon_stop_nrt_profile(str(output_dir).encode())
            print(f"ntff profile: {n} file(s) written to {output_dir}")

    mod = types.ModuleType("antenv.axon_hooks")
    mod.get_axon_ntff_profile_hook = lambda: _hook
    mod.set_axon_ntff_profile_hook = lambda h: None
    import antenv

    sys.modules["antenv.axon_hooks"] = mod
    antenv.axon_hooks = mod

    # keep profile artifacts local instead of uploading to fishnet
    import concourse.bass_utils as _bu

    _bu.upload_artifacts = lambda tmpdir: tmpdir
    return True


LAST_EXEC_NS = None
LAST_RESULTS = None


def kernel(imgsA, recon_A, warped_BA, flow_BA):
    global LAST_EXEC_NS, LAST_RESULTS
    if "nc" not in _CACHE:
        _CACHE["nc"] = _build_program()
    nc = _CACHE["nc"]

    in_maps = _shard_inputs(
        np.asarray(imgsA, np.float32),
        np.asarray(recon_A, np.float32),
        np.asarray(warped_BA, np.float32),
        np.asarray(flow_BA, np.float32),
    )
    trace = os.environ.get("GVSL_TRACE", "0") == "1"
    if trace:
        trace = _install_profile_shim()
    tmpdir = os.environ.get("GVSL_TRACE_DIR") or None
    res = run_bass_kernel_spmd(
        nc, in_maps, core_ids=list(range(8)), trace=trace, tmpdir=tmpdir
    )
    LAST_EXEC_NS = res.exec_time_ns
    LAST_RESULTS = res

    cc = mse = dx = dy = dz = 0.0
    for r in res.results:
        o = np.asarray(r["out"], np.float64)
        cc += o[:, COL_CC0 : COL_CC0 + 4].sum()
        mse += o[:, COL_MSE].sum()
        dx += o[:, COL_DX : COL_DX + 3].sum()
        dy += o[: HP - 1, COL_DY : COL_DY + 24].sum()
        dz += o[:, COL_DZ : COL_DZ + 3].sum()

    n_vox = 2 * 1 * 128 * 128 * 128
    n_d = 2 * 3 * 127 * 128 * 128
    ncc_loss = 1.0 - cc / n_vox
    mse_loss = mse / n_vox
    smooth_loss = (dx / n_d + dy / n_d + dz / n_d) / 3.0
    return (
        np.float32(ncc_loss),
        np.float32(mse_loss),
        np.float32(smooth_loss),
    )
